# revision 30
# baseline (speedup 1.0000x reference)
"""Trainium2 Bass kernel for nn_CFormerAdapter (CIF audio adapter).

Sharding: pure data parallelism — one batch element per NeuronCore (B=8).

Per core:
  alphas  = sigmoid(audio[:, -1])                          # [T]
  pred    = sum(alphas)                                    # scalar output
  alphas *= num_tokens / pred
  W       = CIF integrate-and-fire weights                 # [M, T]
  hT      = audio[:, :-1].T @ W.T                          # [D, M]  (TensorE)
  h2T     = cif_w.T @ hT (+bias via ones row)              # [H, M]  (TensorE)
  hnT     = h2T * rsqrt(mean_d(h2T^2) + 1e-6)              # RMSNorm
  out     = hnT.T @ text_w (+bias row) with rms_w folded   # [M, OUT]

The sequential CIF scan is parallelized exactly via a cumulative sum:
with every alpha < 1 (alpha = sigmoid * scale, scale < 0.6 always here),
integrate_t = C_t - floor(C_t) and the fire count is floor(C_t), so the
two nonzero weights per time step (a_int at token j0, remainder at j1) are
reconstructed from C = cumsum(alphas) with a hardware prefix-scan plus
iota-compare masks.  Numerical deviation from the sequential reference is
bounded by the cumsum rounding drift (~1e-4 absolute on weights).

The CIF einsum runs in float32r (~1.6e-4 rel err, full PE rate, even
moving-dim required); the cif/text projections run in float16 (weights are
~0.1-scale, so fp16 costs ~5e-4 rel err vs bf16's 2.3e-3) which halves the
dominant weight DMA.  rms_w is folded into text_w on the host and the 1/rms
scaling is fused into the text-proj PSUM->SBUF copy (valid because 1/rms
varies along the PSUM partition dim).  DMA is spread across both hardware
DGE queues (sync + scalar engines) plus the gpsimd software queue, weights
are host-prepacked into fully linear per-tile DMAs, and dummy bf16 matmuls
keep the PE's HAM activity monitor at full clock while the audio streams in.
Measured: ~205-210us per core on TRN2, absmax rel err ~6.6e-4 vs the fp32
reference.
"""

import numpy as np
from contextlib import ExitStack

B = 8
T = 1500
HF = 1280
D = HF - 1
M = 375
OUT = 4096
NCH = 12           # time chunks of 128 (11*128 + 92)
TLAST = T - 11 * 128
DCH = 10           # d chunks of the 1279 contraction (9*128 + 127)
KCH = 10           # 1280 = 10*128
MP = 376           # on-chip padded token dim (fp32r needs an even moving dim)
MCH = 3            # 375 = 3*125
MW = 125
NOUT = 8           # 4096 = 8*512


def _emit(nc, tc, tile, mybir, io):
    f32 = mybir.dt.float32
    f32r = mybir.dt.float32r
    f16 = mybir.dt.float16
    i32 = mybir.dt.int32
    Alu = mybir.AluOpType
    Act = mybir.ActivationFunctionType

    audio = io["audio"]
    cifw_d = io["cif_w"]
    tw_d = io["text_w"]
    tb_d = io["text_b"]
    ident_d = io["ident"]
    ustrict_d = io["ustrict"]
    nm1_d = io["nm1col"]
    nt_d = io["nt11"]
    out_h = io["out_h"]
    out_pred = io["out_pred"]

    with ExitStack() as ctx:
        pers = ctx.enter_context(tc.tile_pool(name="pers", bufs=1))
        pp = ctx.enter_context(tc.tile_pool(name="pp", bufs=1, space="PSUM"))

        # ---------------- constants ----------------
        ident = pers.tile([128, 128], f32, name="ident")
        nc.gpsimd.dma_start(ident[:], ident_d[:])
        ustrict = pers.tile([12, 12], f32, name="ustrict")
        nc.gpsimd.dma_start(ustrict[:], ustrict_d[:])
        sdiag = pers.tile([12, 12], f32, name="sdiag")
        nc.gpsimd.dma_start(sdiag[:], io["sdiag"][:])
        nm1col = pers.tile([12, 1], f32, name="nm1col")
        nc.gpsimd.dma_start(nm1col[:], nm1_d[:])
        nt11 = pers.tile([1, 1], f32, name="nt11")
        nc.gpsimd.dma_start(nt11[:], nt_d[:])
        ones128 = pers.tile([128, 1], f32, name="ones128")
        nc.vector.memset(ones128[:], 1.0)
        ones128h = pers.tile([128, 1], f16, name="ones128h")
        nc.vector.tensor_copy(ones128h[:], ones128[:])
        onesrow = pers.tile([1, 128], f32, name="onesrow")
        nc.vector.memset(onesrow[:], 1.0)
        onesrowh = pers.tile([1, 128], f16, name="onesrowh")
        nc.vector.tensor_copy(onesrowh[:], onesrow[:])
        iota_i = pers.tile([128, MP], i32, name="iota_i")
        nc.gpsimd.iota(iota_i[:], pattern=[[1, MP]], base=0, channel_multiplier=0)
        iota_f = pers.tile([128, MP], f32, name="iota_f")
        nc.vector.tensor_copy(iota_f[:], iota_i[:])
        # global time index over the [12, 128] scan layout, and the
        # "not the last step" mask used to kill the t=T-1 remainder
        giota_i = pers.tile([12, 128], i32, name="giota_i")
        nc.gpsimd.iota(giota_i[:], pattern=[[1, 128]], base=0, channel_multiplier=128)
        giota_f = pers.tile([12, 128], f32, name="giota_f")
        nc.vector.tensor_copy(giota_f[:], giota_i[:])
        maskv = pers.tile([12, 128], f32, name="maskv")
        nc.vector.tensor_scalar(
            maskv[:], giota_f[:], float(T - 1), None, op0=Alu.not_equal
        )

        twp = ctx.enter_context(tc.tile_pool(name="twp", bufs=16))
        osb = ctx.enter_context(tc.tile_pool(name="osb", bufs=4))
        tball = pers.tile([1, OUT], f16, name="tball")
        nc.gpsimd.dma_start(tball[:], tb_d[:])

        # persistent results of the einsum / cif stages
        outT = [pers.tile([128, MP], f16, name=f"outT{i}") for i in range(DCH)]
        hnT = [pers.tile([128, MP], f16, name=f"hnT{i}") for i in range(KCH)]
        cifw = [pers.tile([128, HF], f16, name=f"cifw{i}") for i in range(DCH)]

        with tc.tile_pool(name="big", bufs=1) as big:
            hsb = [big.tile([128, HF], f32r, name=f"hsb{i}") for i in range(NCH)]
            wt = [big.tile([128, MP], f32r, name=f"wt{i}") for i in range(NCH)]
            for tcn in range(NCH):
                kk = TLAST if tcn == NCH - 1 else 128
                eng = [nc.sync, nc.scalar, nc.gpsimd, nc.sync, nc.scalar,
                       nc.sync, nc.scalar, nc.gpsimd, nc.sync, nc.scalar,
                       nc.sync, nc.scalar][tcn]
                eng.dma_start(
                    hsb[tcn][0:kk, :], audio[128 * tcn : 128 * tcn + kk, :]
                )

            for dc in range(DCH):
                eng = nc.sync if dc % 2 == 0 else nc.scalar
                eng.dma_start(cifw[dc][:], cifw_d[dc])

            # PE warm-up: keep the HAM activity monitor at full clock while
            # the audio DMA streams in, so the real matmuls start warm.
            warm_ps = pp.tile([128, 512], f32, name="warm_ps", tag="tmp2")
            identB = pers.tile([128, 128], mybir.dt.bfloat16, name="identB")
            nc.vector.tensor_copy(identB[:], ident[:])
            junkB = pers.tile([128, 512], mybir.dt.bfloat16, name="junkB")
            nc.vector.memset(junkB[:], 0.5)

            def warm(n):
                for _ in range(n):
                    nc.tensor.matmul(
                        warm_ps[:], identB[:], junkB[:], start=True, stop=True
                    )

            warm(135)

            # ---------------- alphas ----------------
            araw = pers.tile([128, NCH], f32, name="araw")
            nc.vector.memset(araw[:], 0.0)
            for tcn in range(NCH):
                kk = TLAST if tcn == NCH - 1 else 128
                nc.vector.tensor_copy(
                    araw[0:kk, tcn : tcn + 1],
                    hsb[tcn][0:kk, HF - 1 : HF].bitcast(f32),
                )
            sig = pers.tile([128, NCH], f32, name="sig")
            nc.vector.memset(sig[:], 0.0)
            nc.scalar.activation(sig[:, 0 : NCH - 1], araw[:, 0 : NCH - 1], Act.Sigmoid)
            nc.scalar.activation(
                sig[0:TLAST, NCH - 1 : NCH], araw[0:TLAST, NCH - 1 : NCH], Act.Sigmoid
            )

            rsum = pers.tile([128, 1], f32, name="rsum")
            nc.vector.tensor_reduce(
                rsum[:], sig[:], axis=mybir.AxisListType.X, op=Alu.add
            )
            pred_ps = pp.tile([1, 1], f32, name="pred_ps", tag="tmp")
            nc.tensor.matmul(pred_ps[:], ones128[:], rsum[:], start=True, stop=True)
            pred_sb = pers.tile([1, 1], f32, name="pred_sb")
            nc.vector.tensor_copy(pred_sb[:], pred_ps[:])
            nc.gpsimd.dma_start(out_pred[:], pred_sb[:])
            rp = pers.tile([1, 1], f32, name="rp")
            nc.vector.reciprocal(rp[:], pred_sb[:])
            scale11 = pers.tile([1, 1], f32, name="scale11")
            nc.vector.tensor_mul(scale11[:], rp[:], nt11[:])

            sigT_ps = pp.tile([12, 128], f32, name="sigT_ps", tag="tmp")
            nc.tensor.transpose(sigT_ps[:], sig[:], ident[:])
            scol_ps = pp.tile([12, 1], f32, name="scol_ps", tag="tmp2")
            nc.tensor.matmul(
                scol_ps[:], onesrow[0:1, 0:12], scale11[:], start=True, stop=True
            )
            alph = pers.tile([12, 128], f32, name="alph")
            nc.vector.tensor_scalar(alph[:], sigT_ps[:], scol_ps[:], None, op0=Alu.mult)

            # ---------------- cumsum & fire reconstruction ----------------
            cloc = pers.tile([12, 128], f32, name="cloc")
            nc.vector.tensor_tensor_scan(
                cloc[:], alph[:], alph[:], 0.0, op0=Alu.add, op1=Alu.bypass
            )
            offs_ps = pp.tile([12, 1], f32, name="offs_ps", tag="tmp2")
            nc.tensor.matmul(
                offs_ps[:], ustrict[:], cloc[:, 127:128], start=True, stop=True
            )
            cc = pers.tile([12, 128], f32, name="cc")
            nc.vector.tensor_scalar(cc[:], cloc[:], offs_ps[:], None, op0=Alu.add)

            kint = pers.tile([12, 128], i32, name="kint")
            nc.vector.tensor_copy(kint[:], cc[:])
            kf = pers.tile([12, 128], f32, name="kf")
            nc.vector.tensor_copy(kf[:], kint[:])
            kgt = pers.tile([12, 128], f32, name="kgt")
            nc.vector.tensor_tensor(kgt[:], kf[:], cc[:], op=Alu.is_gt)
            kk_t = pers.tile([12, 128], f32, name="kk_t")
            nc.vector.tensor_tensor(kk_t[:], kf[:], kgt[:], op=Alu.subtract)
            frac = pers.tile([12, 128], f32, name="frac")
            nc.vector.tensor_tensor(frac[:], cc[:], kk_t[:], op=Alu.subtract)

            # cross-partition shift of the last column via superdiagonal matmul
            kprev = pers.tile([12, 128], f32, name="kprev")
            nc.vector.tensor_copy(kprev[:, 1:128], kk_t[:, 0:127])
            ksh_ps = pp.tile([12, 1], f32, name="ksh_ps", tag="tmp2")
            nc.tensor.matmul(ksh_ps[:], sdiag[:], kk_t[:, 127:128], start=True, stop=True)
            nc.vector.tensor_copy(kprev[:, 0:1], ksh_ps[:])
            fprev = pers.tile([12, 128], f32, name="fprev")
            nc.vector.tensor_copy(fprev[:, 1:128], frac[:, 0:127])
            fsh_ps = pp.tile([12, 1], f32, name="fsh_ps", tag="tmp2")
            nc.tensor.matmul(fsh_ps[:], sdiag[:], frac[:, 127:128], start=True, stop=True)
            nc.vector.tensor_copy(fprev[:, 0:1], fsh_ps[:])

            fired = pers.tile([12, 128], f32, name="fired")
            nc.vector.tensor_tensor(fired[:], kk_t[:], kprev[:], op=Alu.is_gt)

            # pack rows (32-aligned blocks): 0-11 a_int, 32-43 rem, 64-75 j0, 96-107 j1
            pack = pers.tile([128, 128], f32, name="pack")
            nc.vector.memset(pack[:], 0.0)
            afired = pers.tile([12, 128], f32, name="afired")
            nc.vector.tensor_scalar(
                afired[:], fprev[:], -1.0, 1.0, op0=Alu.mult, op1=Alu.add
            )
            d0 = pers.tile([12, 128], f32, name="d0")
            nc.vector.tensor_tensor(d0[:], afired[:], alph[:], op=Alu.subtract)
            t0 = pers.tile([12, 128], f32, name="t0")
            nc.vector.tensor_tensor(t0[:], d0[:], fired[:], op=Alu.mult)
            nc.vector.tensor_tensor(pack[0:12, :], t0[:], alph[:], op=Alu.add)
            remv = pers.tile([12, 128], f32, name="remv")
            nc.vector.tensor_tensor(remv[:], alph[:], pack[0:12, :], op=Alu.subtract)
            # t = T-1 never writes a remainder -> mask it out
            nc.vector.tensor_tensor(pack[32:44, :], remv[:], maskv[:], op=Alu.mult)
            nc.vector.tensor_scalar(
                pack[64:76, :], kprev[:], nm1col[:], None, op0=Alu.min
            )
            nc.vector.tensor_scalar(
                pack[96:108, :], kk_t[:], nm1col[:], None, op0=Alu.min
            )

            packT_ps = pp.tile([128, 128], f32, name="packT_ps", tag="tmp")
            nc.tensor.transpose(packT_ps[:], pack[:], ident[:])
            pk = pers.tile([128, 128], f32, name="pk")
            nc.vector.tensor_copy(pk[:], packT_ps[:])

            # ---------------- W build (per time chunk) ----------------
            with tc.tile_pool(name="wtmp", bufs=2) as wtmp:
                for tcn in range(NCH):
                    ve = nc.vector
                    w0 = wtmp.tile([128, MP], f32, name="w0", tag="w0")
                    ve.tensor_scalar(
                        w0[:], iota_f[:],
                        pk[:, 64 + tcn : 65 + tcn], pk[:, tcn : tcn + 1],
                        op0=Alu.is_equal, op1=Alu.mult,
                    )
                    w1 = wtmp.tile([128, MP], f32, name="w1", tag="w1")
                    ve.tensor_scalar(
                        w1[:], iota_f[:],
                        pk[:, 96 + tcn : 97 + tcn], pk[:, 32 + tcn : 33 + tcn],
                        op0=Alu.is_equal, op1=Alu.mult,
                    )
                    ve.tensor_tensor(wt[tcn][:], w0[:], w1[:], op=Alu.add)

            warm(12)
            # ---------------- CIF einsum: outT[d, m] = sum_t H[t, d] W[m, t] ----
            with tc.tile_pool(name="mmps", bufs=1, space="PSUM") as mmps:
                for g in range(2):
                    ps_mm = [
                        mmps.tile([128, MP], f32, name=f"mm{i}", tag=f"mm{i}")
                        for i in range(5)
                    ]
                    for tcn in range(NCH):
                        kk = TLAST if tcn == NCH - 1 else 128
                        for i in range(5):
                            dc = 5 * g + i
                            dw = 127 if dc == 9 else 128
                            nc.tensor.matmul(
                                ps_mm[i][0:dw, :],
                                hsb[tcn][0:kk, 128 * dc : 128 * dc + dw],
                                wt[tcn][0:kk, :],
                                start=(tcn == 0),
                                stop=(tcn == NCH - 1),
                            )
                    for i in range(5):
                        dc = 5 * g + i
                        dw = 127 if dc == 9 else 128
                        nc.vector.tensor_copy(outT[dc][0:dw, :], ps_mm[i][0:dw, :])
            # ones row for the cif bias trick (DMA: engines can't start at p=127)
            nc.gpsimd.dma_start(outT[9][127:128, :], io["onesM"][:])

        warm(10)
        # ---------------- cif projection (transposed) + RMS ----------------
        if True:
            with (
                tc.tile_pool(name="cps", bufs=2, space="PSUM") as cps,
                tc.tile_pool(name="cps1", bufs=1, space="PSUM") as cps1,
                tc.tile_pool(name="sqp", bufs=1) as sqp,
            ):
                sqts = []
                ssq_ps = cps1.tile([1, MP], f32, name="ssq_ps", tag="ssq")
                for h2 in range(KCH):
                    ps2 = cps.tile([128, MP], f32, name="ps2", tag="ps2")
                    for dc in range(DCH):
                        nc.tensor.matmul(
                            ps2[:],
                            cifw[dc][:, 128 * h2 : 128 * h2 + 128],
                            outT[dc][:],
                            start=(dc == 0),
                            stop=(dc == DCH - 1),
                        )
                    nc.vector.tensor_copy(hnT[h2][:], ps2[:])
                    sqt = sqp.tile([128, MP], f16, name=f"sqt{h2}", tag=f"sqt{h2}")
                    nc.vector.tensor_tensor(sqt[:], hnT[h2][:], hnT[h2][:], op=Alu.mult)
                    sqts.append(sqt)
                # ssq matmuls emitted after all cif matmuls: the PE stream is
                # not head-of-line blocked waiting on the DVE squares
                for h2 in range(KCH):
                    nc.tensor.matmul(
                        ssq_ps[:], ones128h[:], sqts[h2][:],
                        start=(h2 == 0), stop=(h2 == KCH - 1),
                    )

                eps11 = pers.tile([1, 1], f32, name="eps11")
                nc.vector.memset(eps11[:], 1e-6)
                sqrtv = pers.tile([1, MP], f32, name="sqrtv")
                nc.scalar.activation(
                    sqrtv[:], ssq_ps[:], Act.Sqrt, bias=eps11[:], scale=1.0 / HF
                )
                ones11 = pers.tile([1, 1], f32, name="ones11")
                nc.vector.memset(ones11[:], 1.0)

        # ---------------- text projection ----------------
        tbrep = pers.tile([128, OUT], f16, name="tbrep")
        with tc.tile_pool(name="tps", bufs=1, space="PSUM") as tps:
            for jj in range(NOUT):
                tb_ps = pp.tile([128, 512], f32, name=f"tb_ps{jj}", tag="tmp")
                nc.tensor.matmul(
                    tb_ps[:], onesrowh[:], tball[0:1, 512 * jj : 512 * jj + 512],
                    start=True, stop=True,
                )
                nc.vector.tensor_copy(tbrep[:, 512 * jj : 512 * jj + 512], tb_ps[:])
            # n-blocks of 1024 so text_w DMAs move 4KB contiguous lines
            rinvT = []
            for nb in range(NOUT // 2):
                warm(4)
                ps3 = [
                    [
                        tps.tile([MW, 512], f32, name=f"ps3_{m}_{j}", tag=f"ps3_{m}_{j}")
                        for j in range(2)
                    ]
                    for m in range(MCH)
                ]
                for k in range(KCH):
                    twt = twp.tile([128, 1024], f16, name="twt", tag="twt")
                    eng = nc.sync if k % 2 == 0 else nc.scalar
                    eng.dma_start(twt[:], tw_d[nb * KCH + k])
                    for m in range(MCH):
                        for j in range(2):
                            nc.tensor.matmul(
                                ps3[m][j][:],
                                hnT[k][:, MW * m : MW * m + MW],
                                twt[:, 512 * j : 512 * j + 512],
                                start=(k == 0),
                                stop=(k == KCH - 1),
                            )
                if nb == 0:
                    # transposed 1/rms columns, emitted here so the PE stream
                    # reaches them only after independent text matmuls (the
                    # ACT sqrt they depend on finishes in the meantime)
                    for m in range(MCH):
                        rt_ps = pp.tile([MW, 1], f32, name=f"rt_ps{m}", tag="tmp2")
                        nc.tensor.matmul(
                            rt_ps[:], sqrtv[0:1, MW * m : MW * m + MW], ones11[:],
                            start=True, stop=True,
                        )
                        rt = pers.tile([MW, 1], f32, name=f"rinvT{m}")
                        nc.vector.reciprocal(rt[:], rt_ps[:])
                        rinvT.append(rt)
                for m in range(MCH):
                    for j in range(2):
                        n = 2 * nb + j
                        ot = osb.tile([MW, 512], f32, name="ot", tag="ot")
                        nc.vector.scalar_tensor_tensor(
                            ot[:], ps3[m][j][:], rinvT[m][:],
                            tbrep[0:MW, 512 * n : 512 * n + 512],
                            op0=Alu.mult, op1=Alu.add,
                        )
                        if nb == 3:
                            oeng = nc.sync if (m + j) % 2 == 0 else nc.scalar
                        else:
                            oeng = (nc.gpsimd, nc.gpsimd, nc.sync)[nb]
                        oeng.dma_start(
                            out_h[MW * m : MW * m + MW, 512 * n : 512 * n + 512],
                            ot[:],
                        )


def build_nc():
    import concourse.tile as tile
    from concourse import bacc, mybir

    f32 = mybir.dt.float32
    f32r = mybir.dt.float32r
    f16 = mybir.dt.float16
    nc = bacc.Bacc(
        "TRN2", target_bir_lowering=False, debug=False, enable_asserts=False
    )
    io = {
        "audio": nc.dram_tensor("audio", [T, HF], f32r, kind="ExternalInput").ap(),
        "cif_w": nc.dram_tensor("cif_w", [DCH, 128, HF], f16, kind="ExternalInput").ap(),
        
        "text_w": nc.dram_tensor("text_w", [NOUT // 2 * KCH, 128, 1024], f16, kind="ExternalInput").ap(),
        "text_b": nc.dram_tensor("text_b", [1, OUT], f16, kind="ExternalInput").ap(),
        "ident": nc.dram_tensor("ident", [128, 128], f32, kind="ExternalInput").ap(),
        "ustrict": nc.dram_tensor("ustrict", [12, 12], f32, kind="ExternalInput").ap(),
        "sdiag": nc.dram_tensor("sdiag", [12, 12], f32, kind="ExternalInput").ap(),
        "nm1col": nc.dram_tensor("nm1col", [12, 1], f32, kind="ExternalInput").ap(),
        "nt11": nc.dram_tensor("nt11", [1, 1], f32, kind="ExternalInput").ap(),
        "onesM": nc.dram_tensor("onesM", [1, MP], f16, kind="ExternalInput").ap(),
        "out_h": nc.dram_tensor("out_h", [M, OUT], f32, kind="ExternalOutput").ap(),
        "out_pred": nc.dram_tensor("out_pred", [1, 1], f32, kind="ExternalOutput").ap(),
    }
    with tile.TileContext(nc) as tc:
        _emit(nc, tc, tile, mybir, io)
    nc.compile()
    return nc


_NC_CACHE = {}


def make_in_maps(audio_features, num_tokens, cif_w, cif_b, text_w_scaled, text_b):
    ident = np.eye(128, dtype=np.float32)
    ustrict = np.triu(np.ones((12, 12), np.float32), k=1)
    sdiag = np.diag(np.ones(11, np.float32), k=1)

    # prepack cif_w (+bias as last row) into [DCH, 128, HF] fp16 tiles
    cifw_p = np.zeros((DCH, 128, HF), np.float16)
    cw16 = cif_w.astype(np.float16)
    for dc in range(DCH - 1):
        cifw_p[dc] = cw16[128 * dc : 128 * dc + 128]
    cifw_p[DCH - 1, 0:127] = cw16[1152:1279]
    cifw_p[DCH - 1, 127] = cif_b.astype(np.float16)

    # prepack text_w into [NOUT//2 * KCH, 128, 1024] fp16 tiles (linear DMAs)
    tw16 = text_w_scaled.astype(np.float16)
    tw_p = np.zeros((NOUT // 2 * KCH, 128, 1024), np.float16)
    for nb in range(NOUT // 2):
        for k in range(KCH):
            tw_p[nb * KCH + k] = tw16[
                128 * k : 128 * k + 128, 1024 * nb : 1024 * nb + 1024
            ]

    in_maps = []
    for b in range(B):
        nt = np.float32(num_tokens[b])
        in_maps.append(
            {
                "audio": np.ascontiguousarray(audio_features[b]),
                "cif_w": cifw_p,
                "text_w": tw_p,
                "text_b": text_b.astype(np.float16).reshape(1, OUT),
                "ident": ident,
                "ustrict": ustrict,
                "sdiag": sdiag,
                "nm1col": np.full((12, 1), nt - 1, np.float32),
                "nt11": np.full((1, 1), nt, np.float32),
                "onesM": np.ones((1, MP), np.float16),
            }
        )
    return in_maps


def kernel(audio_features, num_tokens, rms_w, cif_w, cif_b, text_w, text_b, max_tokens):
    from concourse.bass_utils import run_bass_kernel_spmd

    audio_features = np.asarray(audio_features, dtype=np.float32)
    num_tokens = np.asarray(num_tokens)
    assert int(max_tokens) == M and audio_features.shape == (B, T, HF)

    # fold rms_w into text_w (pure reassociation of (h/rms*rms_w) @ text_w)
    text_w_scaled = (
        np.asarray(text_w, np.float32) * np.asarray(rms_w, np.float32)[:, None]
    ).astype(np.float32)

    if "nc" not in _NC_CACHE:
        _NC_CACHE["nc"] = build_nc()
    nc = _NC_CACHE["nc"]

    in_maps = make_in_maps(
        audio_features, num_tokens,
        np.asarray(cif_w, np.float32), np.asarray(cif_b, np.float32),
        text_w_scaled, np.asarray(text_b, np.float32),
    )
    res = run_bass_kernel_spmd(nc, in_maps, core_ids=list(range(B)))
    h = np.stack([r["out_h"] for r in res.results], axis=0)
    pred = np.array([r["out_pred"][0, 0] for r in res.results], dtype=np.float32)
    return h, pred


# revision 34
# speedup vs baseline: 1.1076x; 1.1076x over previous
"""Trainium2 Bass kernel for nn_CFormerAdapter (CIF audio adapter).

Sharding: pure data parallelism — one batch element per NeuronCore (B=8).

Per core:
  alphas  = sigmoid(audio[:, -1])                          # [T]
  pred    = sum(alphas)                                    # scalar output
  alphas *= num_tokens / pred
  W       = CIF integrate-and-fire weights                 # [M, T]
  hT      = audio[:, :-1].T @ W.T                          # [D, M]  (TensorE)
  h2T     = cif_w.T @ hT (+bias via ones row)              # [H, M]  (TensorE)
  hnT     = h2T * rsqrt(mean_d(h2T^2) + 1e-6)              # RMSNorm
  out     = hnT.T @ text_w (+bias row) with rms_w folded   # [M, OUT]

The sequential CIF scan is parallelized exactly via a cumulative sum:
with every alpha < 1 (alpha = sigmoid * scale, scale < 0.6 always here),
integrate_t = C_t - floor(C_t) and the fire count is floor(C_t), so the
two nonzero weights per time step (a_int at token j0, remainder at j1) are
reconstructed from C = cumsum(alphas) with a hardware prefix-scan plus
iota-compare masks.  Numerical deviation from the sequential reference is
bounded by the cumsum rounding drift (~1e-4 absolute on weights).

The CIF einsum runs in float32r (~1.6e-4 rel err, full PE rate, even
moving-dim required); the cif/text projections run in float16 (weights are
~0.1-scale, so fp16 costs ~5e-4 rel err vs bf16's 2.3e-3) which halves the
dominant weight DMA.  rms_w is folded into text_w on the host and the 1/rms
scaling is fused into the text-proj PSUM->SBUF copy (valid because 1/rms
varies along the PSUM partition dim).  DMA is spread across both hardware
DGE queues (sync + scalar engines) plus the gpsimd software queue, weights
are host-prepacked into fully linear per-tile DMAs, and dummy bf16 matmuls
keep the PE's HAM activity monitor at full clock while the audio streams in.
Measured: ~204-208us per core on TRN2, absmax rel err ~6.6e-4 vs the fp32
reference.
"""

import numpy as np
from contextlib import ExitStack

B = 8
T = 1500
HF = 1280
D = HF - 1
M = 375
OUT = 4096
NCH = 12           # time chunks of 128 (11*128 + 92)
TLAST = T - 11 * 128
DCH = 10           # d chunks of the 1279 contraction (9*128 + 127)
KCH = 10           # 1280 = 10*128
MP = 376           # on-chip padded token dim (fp32r needs an even moving dim)
MCH = 3            # 375 = 3*125
MW = 125
NOUT = 8           # 4096 = 8*512


def _emit(nc, tc, tile, mybir, io):
    f32 = mybir.dt.float32
    f32r = mybir.dt.float32r
    f16 = mybir.dt.float16
    i32 = mybir.dt.int32
    Alu = mybir.AluOpType
    Act = mybir.ActivationFunctionType

    audio = io["audio"]
    cifw_d = io["cif_w"]
    tw_d = io["text_w"]
    tb_d = io["text_b"]
    ident_d = io["ident"]
    ustrict_d = io["ustrict"]
    nm1_d = io["nm1col"]
    nt_d = io["nt11"]
    out_h = io["out_h"]
    out_pred = io["out_pred"]

    with ExitStack() as ctx:
        pers = ctx.enter_context(tc.tile_pool(name="pers", bufs=1))
        pp = ctx.enter_context(tc.tile_pool(name="pp", bufs=1, space="PSUM"))

        # ---------------- constants ----------------
        ident = pers.tile([128, 128], f32, name="ident")
        nc.gpsimd.dma_start(ident[:], ident_d[:])
        ustrict = pers.tile([12, 12], f32, name="ustrict")
        nc.gpsimd.dma_start(ustrict[:], ustrict_d[:])
        sdiag = pers.tile([12, 12], f32, name="sdiag")
        nc.gpsimd.dma_start(sdiag[:], io["sdiag"][:])
        nm1col = pers.tile([12, 1], f32, name="nm1col")
        nc.gpsimd.dma_start(nm1col[:], nm1_d[:])
        nt11 = pers.tile([1, 1], f32, name="nt11")
        nc.gpsimd.dma_start(nt11[:], nt_d[:])
        ones128 = pers.tile([128, 1], f32, name="ones128")
        nc.vector.memset(ones128[:], 1.0)
        ones128h = pers.tile([128, 1], f16, name="ones128h")
        nc.vector.tensor_copy(ones128h[:], ones128[:])
        onesrow = pers.tile([1, 128], f32, name="onesrow")
        nc.vector.memset(onesrow[:], 1.0)
        onesrowh = pers.tile([1, 128], f16, name="onesrowh")
        nc.vector.tensor_copy(onesrowh[:], onesrow[:])
        iota_i = pers.tile([128, MP], i32, name="iota_i")
        nc.gpsimd.iota(iota_i[:], pattern=[[1, MP]], base=0, channel_multiplier=0)
        iota_f = pers.tile([128, MP], f32, name="iota_f")
        nc.vector.tensor_copy(iota_f[:], iota_i[:])
        # global time index over the [12, 128] scan layout, and the
        # "not the last step" mask used to kill the t=T-1 remainder
        giota_i = pers.tile([12, 128], i32, name="giota_i")
        nc.gpsimd.iota(giota_i[:], pattern=[[1, 128]], base=0, channel_multiplier=128)
        giota_f = pers.tile([12, 128], f32, name="giota_f")
        nc.vector.tensor_copy(giota_f[:], giota_i[:])
        maskv = pers.tile([12, 128], f32, name="maskv")
        nc.vector.tensor_scalar(
            maskv[:], giota_f[:], float(T - 1), None, op0=Alu.not_equal
        )

        twp = ctx.enter_context(tc.tile_pool(name="twp", bufs=16))
        osb = ctx.enter_context(tc.tile_pool(name="osb", bufs=6))
        tball = pers.tile([1, OUT], f16, name="tball")
        nc.gpsimd.dma_start(tball[:], tb_d[:])

        # persistent results of the einsum / cif stages
        outT = [pers.tile([128, MP], f16, name=f"outT{i}") for i in range(DCH)]
        hnT = [pers.tile([128, MP], f16, name=f"hnT{i}") for i in range(KCH)]
        cifw = [pers.tile([128, HF], f16, name=f"cifw{i}") for i in range(DCH)]

        with tc.tile_pool(name="big", bufs=1) as big:
            hsb = [big.tile([128, HF], f32r, name=f"hsb{i}") for i in range(NCH)]
            wt = [big.tile([128, MP], f32r, name=f"wt{i}") for i in range(NCH)]
            for tcn in range(NCH):
                kk = TLAST if tcn == NCH - 1 else 128
                eng = [nc.sync, nc.scalar, nc.gpsimd, nc.sync, nc.scalar,
                       nc.sync, nc.scalar, nc.gpsimd, nc.sync, nc.scalar,
                       nc.sync, nc.scalar][tcn]
                eng.dma_start(
                    hsb[tcn][0:kk, :], audio[128 * tcn : 128 * tcn + kk, :]
                )

            for dc in range(DCH):
                eng = nc.scalar if dc % 3 == 2 else nc.sync
                eng.dma_start(cifw[dc][:], cifw_d[dc])

            # PE warm-up: keep the HAM activity monitor at full clock while
            # the audio DMA streams in, so the real matmuls start warm.
            warm_ps = pp.tile([128, 512], f32, name="warm_ps", tag="tmp2")
            identB = pers.tile([128, 128], mybir.dt.bfloat16, name="identB")
            nc.vector.tensor_copy(identB[:], ident[:])
            junkB = pers.tile([128, 512], mybir.dt.bfloat16, name="junkB")
            nc.vector.memset(junkB[:], 0.5)

            def warm(n):
                for _ in range(n):
                    nc.tensor.matmul(
                        warm_ps[:], identB[:], junkB[:], start=True, stop=True
                    )

            warm(135)

            # ---------------- alphas ----------------
            araw = pers.tile([128, NCH], f32, name="araw")
            nc.vector.memset(araw[:], 0.0)
            for tcn in range(NCH):
                kk = TLAST if tcn == NCH - 1 else 128
                nc.vector.tensor_copy(
                    araw[0:kk, tcn : tcn + 1],
                    hsb[tcn][0:kk, HF - 1 : HF].bitcast(f32),
                )
            sig = pers.tile([128, NCH], f32, name="sig")
            nc.vector.memset(sig[:], 0.0)
            nc.scalar.activation(sig[:, 0 : NCH - 1], araw[:, 0 : NCH - 1], Act.Sigmoid)
            nc.scalar.activation(
                sig[0:TLAST, NCH - 1 : NCH], araw[0:TLAST, NCH - 1 : NCH], Act.Sigmoid
            )

            rsum = pers.tile([128, 1], f32, name="rsum")
            nc.vector.tensor_reduce(
                rsum[:], sig[:], axis=mybir.AxisListType.X, op=Alu.add
            )
            pred_ps = pp.tile([1, 1], f32, name="pred_ps", tag="tmp")
            nc.tensor.matmul(pred_ps[:], ones128[:], rsum[:], start=True, stop=True)
            pred_sb = pers.tile([1, 1], f32, name="pred_sb")
            nc.vector.tensor_copy(pred_sb[:], pred_ps[:])
            nc.gpsimd.dma_start(out_pred[:], pred_sb[:])
            rp = pers.tile([1, 1], f32, name="rp")
            nc.vector.reciprocal(rp[:], pred_sb[:])
            scale11 = pers.tile([1, 1], f32, name="scale11")
            nc.vector.tensor_mul(scale11[:], rp[:], nt11[:])

            sigT_ps = pp.tile([12, 128], f32, name="sigT_ps", tag="tmp")
            nc.tensor.transpose(sigT_ps[:], sig[:], ident[:])
            scol_ps = pp.tile([12, 1], f32, name="scol_ps", tag="tmp2")
            nc.tensor.matmul(
                scol_ps[:], onesrow[0:1, 0:12], scale11[:], start=True, stop=True
            )
            alph = pers.tile([12, 128], f32, name="alph")
            nc.vector.tensor_scalar(alph[:], sigT_ps[:], scol_ps[:], None, op0=Alu.mult)

            # ---------------- cumsum & fire reconstruction ----------------
            cloc = pers.tile([12, 128], f32, name="cloc")
            nc.vector.tensor_tensor_scan(
                cloc[:], alph[:], alph[:], 0.0, op0=Alu.add, op1=Alu.bypass
            )
            offs_ps = pp.tile([12, 1], f32, name="offs_ps", tag="tmp2")
            nc.tensor.matmul(
                offs_ps[:], ustrict[:], cloc[:, 127:128], start=True, stop=True
            )
            cc = pers.tile([12, 128], f32, name="cc")
            nc.vector.tensor_scalar(cc[:], cloc[:], offs_ps[:], None, op0=Alu.add)

            kint = pers.tile([12, 128], i32, name="kint")
            nc.vector.tensor_copy(kint[:], cc[:])
            kf = pers.tile([12, 128], f32, name="kf")
            nc.vector.tensor_copy(kf[:], kint[:])
            kgt = pers.tile([12, 128], f32, name="kgt")
            nc.vector.tensor_tensor(kgt[:], kf[:], cc[:], op=Alu.is_gt)
            kk_t = pers.tile([12, 128], f32, name="kk_t")
            nc.vector.tensor_tensor(kk_t[:], kf[:], kgt[:], op=Alu.subtract)
            frac = pers.tile([12, 128], f32, name="frac")
            nc.vector.tensor_tensor(frac[:], cc[:], kk_t[:], op=Alu.subtract)

            # cross-partition shift of the last column via superdiagonal matmul
            kprev = pers.tile([12, 128], f32, name="kprev")
            nc.vector.tensor_copy(kprev[:, 1:128], kk_t[:, 0:127])
            ksh_ps = pp.tile([12, 1], f32, name="ksh_ps", tag="tmp2")
            nc.tensor.matmul(ksh_ps[:], sdiag[:], kk_t[:, 127:128], start=True, stop=True)
            nc.vector.tensor_copy(kprev[:, 0:1], ksh_ps[:])
            fprev = pers.tile([12, 128], f32, name="fprev")
            nc.vector.tensor_copy(fprev[:, 1:128], frac[:, 0:127])
            fsh_ps = pp.tile([12, 1], f32, name="fsh_ps", tag="tmp2")
            nc.tensor.matmul(fsh_ps[:], sdiag[:], frac[:, 127:128], start=True, stop=True)
            nc.vector.tensor_copy(fprev[:, 0:1], fsh_ps[:])

            fired = pers.tile([12, 128], f32, name="fired")
            nc.vector.tensor_tensor(fired[:], kk_t[:], kprev[:], op=Alu.is_gt)

            # pack rows (32-aligned blocks): 0-11 a_int, 32-43 rem, 64-75 j0, 96-107 j1
            pack = pers.tile([128, 128], f32, name="pack")
            nc.vector.memset(pack[:], 0.0)
            afired = pers.tile([12, 128], f32, name="afired")
            nc.vector.tensor_scalar(
                afired[:], fprev[:], -1.0, 1.0, op0=Alu.mult, op1=Alu.add
            )
            d0 = pers.tile([12, 128], f32, name="d0")
            nc.vector.tensor_tensor(d0[:], afired[:], alph[:], op=Alu.subtract)
            t0 = pers.tile([12, 128], f32, name="t0")
            nc.vector.tensor_tensor(t0[:], d0[:], fired[:], op=Alu.mult)
            nc.vector.tensor_tensor(pack[0:12, :], t0[:], alph[:], op=Alu.add)
            remv = pers.tile([12, 128], f32, name="remv")
            nc.vector.tensor_tensor(remv[:], alph[:], pack[0:12, :], op=Alu.subtract)
            # t = T-1 never writes a remainder -> mask it out
            nc.vector.tensor_tensor(pack[32:44, :], remv[:], maskv[:], op=Alu.mult)
            nc.vector.tensor_scalar(
                pack[64:76, :], kprev[:], nm1col[:], None, op0=Alu.min
            )
            nc.vector.tensor_scalar(
                pack[96:108, :], kk_t[:], nm1col[:], None, op0=Alu.min
            )

            packT_ps = pp.tile([128, 128], f32, name="packT_ps", tag="tmp")
            nc.tensor.transpose(packT_ps[:], pack[:], ident[:])
            pk = pers.tile([128, 128], f32, name="pk")
            nc.vector.tensor_copy(pk[:], packT_ps[:])

            # ---------------- W build (per time chunk) ----------------
            with tc.tile_pool(name="wtmp", bufs=2) as wtmp:
                for tcn in range(NCH):
                    ve = nc.vector
                    w0 = wtmp.tile([128, MP], f32, name="w0", tag="w0")
                    ve.tensor_scalar(
                        w0[:], iota_f[:],
                        pk[:, 64 + tcn : 65 + tcn], pk[:, tcn : tcn + 1],
                        op0=Alu.is_equal, op1=Alu.mult,
                    )
                    w1 = wtmp.tile([128, MP], f32, name="w1", tag="w1")
                    ve.tensor_scalar(
                        w1[:], iota_f[:],
                        pk[:, 96 + tcn : 97 + tcn], pk[:, 32 + tcn : 33 + tcn],
                        op0=Alu.is_equal, op1=Alu.mult,
                    )
                    ve.tensor_tensor(wt[tcn][:], w0[:], w1[:], op=Alu.add)

            warm(12)
            # ---------------- CIF einsum: outT[d, m] = sum_t H[t, d] W[m, t] ----
            with tc.tile_pool(name="mmps", bufs=1, space="PSUM") as mmps:
                for g in range(2):
                    ps_mm = [
                        mmps.tile([128, MP], f32, name=f"mm{i}", tag=f"mm{i}")
                        for i in range(5)
                    ]
                    for tcn in range(NCH):
                        kk = TLAST if tcn == NCH - 1 else 128
                        for i in range(5):
                            dc = 5 * g + i
                            dw = 127 if dc == 9 else 128
                            nc.tensor.matmul(
                                ps_mm[i][0:dw, :],
                                hsb[tcn][0:kk, 128 * dc : 128 * dc + dw],
                                wt[tcn][0:kk, :],
                                start=(tcn == 0),
                                stop=(tcn == NCH - 1),
                            )
                    for i in range(5):
                        dc = 5 * g + i
                        dw = 127 if dc == 9 else 128
                        nc.vector.tensor_copy(outT[dc][0:dw, :], ps_mm[i][0:dw, :])
            # ones row for the cif bias trick (DMA: engines can't start at p=127)
            nc.gpsimd.dma_start(outT[9][127:128, :], io["onesM"][:])

        warm(10)
        # ---------------- cif projection (transposed) + RMS ----------------
        if True:
            with (
                tc.tile_pool(name="cps", bufs=2, space="PSUM") as cps,
                tc.tile_pool(name="cps1", bufs=1, space="PSUM") as cps1,
                tc.tile_pool(name="sqp", bufs=1) as sqp,
            ):
                sqts = []
                ssq_ps = cps1.tile([1, MP], f32, name="ssq_ps", tag="ssq")
                for h2 in range(KCH):
                    ps2 = cps.tile([128, MP], f32, name="ps2", tag="ps2")
                    for dc in range(DCH):
                        nc.tensor.matmul(
                            ps2[:],
                            cifw[dc][:, 128 * h2 : 128 * h2 + 128],
                            outT[dc][:],
                            start=(dc == 0),
                            stop=(dc == DCH - 1),
                        )
                    nc.vector.tensor_copy(hnT[h2][:], ps2[:])
                    sqt = sqp.tile([128, MP], f16, name=f"sqt{h2}", tag=f"sqt{h2}")
                    nc.vector.tensor_tensor(sqt[:], hnT[h2][:], hnT[h2][:], op=Alu.mult)
                    sqts.append(sqt)
                # ssq matmuls emitted after all cif matmuls: the PE stream is
                # not head-of-line blocked waiting on the DVE squares
                for h2 in range(KCH):
                    nc.tensor.matmul(
                        ssq_ps[:], ones128h[:], sqts[h2][:],
                        start=(h2 == 0), stop=(h2 == KCH - 1),
                    )

                eps11 = pers.tile([1, 1], f32, name="eps11")
                nc.vector.memset(eps11[:], 1e-6)
                sqrtv = pers.tile([1, MP], f32, name="sqrtv")
                nc.scalar.activation(
                    sqrtv[:], ssq_ps[:], Act.Sqrt, bias=eps11[:], scale=1.0 / HF
                )
                ones11 = pers.tile([1, 1], f32, name="ones11")
                nc.vector.memset(ones11[:], 1.0)

        # ---------------- text projection ----------------
        tbrep = pers.tile([128, OUT], f16, name="tbrep")
        with tc.tile_pool(name="tps", bufs=1, space="PSUM") as tps:
            for jj in range(NOUT):
                tb_ps = pp.tile([128, 512], f32, name=f"tb_ps{jj}", tag="tmp")
                nc.tensor.matmul(
                    tb_ps[:], onesrowh[:], tball[0:1, 512 * jj : 512 * jj + 512],
                    start=True, stop=True,
                )
                nc.vector.tensor_copy(tbrep[:, 512 * jj : 512 * jj + 512], tb_ps[:])
            # n-blocks of 1024 so text_w DMAs move 4KB contiguous lines
            rinvT = []
            for nb in range(NOUT // 2):
                warm(4)
                ps3 = [
                    [
                        tps.tile([MW, 512], f32, name=f"ps3_{m}_{j}", tag=f"ps3_{m}_{j}")
                        for j in range(2)
                    ]
                    for m in range(MCH)
                ]
                for k in range(KCH):
                    twt = twp.tile([128, 1024], f16, name="twt", tag="twt")
                    eng = nc.sync if k % 2 == 0 else nc.scalar
                    eng.dma_start(twt[:], tw_d[nb * KCH + k])
                    for m in range(MCH):
                        for j in range(2):
                            nc.tensor.matmul(
                                ps3[m][j][:],
                                hnT[k][:, MW * m : MW * m + MW],
                                twt[:, 512 * j : 512 * j + 512],
                                start=(k == 0),
                                stop=(k == KCH - 1),
                            )
                if nb == 0:
                    # transposed 1/rms columns, emitted here so the PE stream
                    # reaches them only after independent text matmuls (the
                    # ACT sqrt they depend on finishes in the meantime)
                    for m in range(MCH):
                        rt_ps = pp.tile([MW, 1], f32, name=f"rt_ps{m}", tag="tmp2")
                        nc.tensor.matmul(
                            rt_ps[:], sqrtv[0:1, MW * m : MW * m + MW], ones11[:],
                            start=True, stop=True,
                        )
                        rt = pers.tile([MW, 1], f32, name=f"rinvT{m}")
                        nc.vector.reciprocal(rt[:], rt_ps[:])
                        rinvT.append(rt)
                for m in range(MCH):
                    for j in range(2):
                        n = 2 * nb + j
                        ot = osb.tile([MW, 512], f32, name="ot", tag="ot")
                        nc.vector.scalar_tensor_tensor(
                            ot[:], ps3[m][j][:], rinvT[m][:],
                            tbrep[0:MW, 512 * n : 512 * n + 512],
                            op0=Alu.mult, op1=Alu.add,
                        )
                        if nb == 3:
                            oeng = nc.sync if (m + j) % 2 == 0 else nc.scalar
                        else:
                            oeng = (nc.gpsimd, nc.gpsimd, nc.sync)[nb]
                        oeng.dma_start(out_h[6 * nb + 2 * m + j], ot[:])


def build_nc():
    import concourse.tile as tile
    from concourse import bacc, mybir

    f32 = mybir.dt.float32
    f32r = mybir.dt.float32r
    f16 = mybir.dt.float16
    nc = bacc.Bacc(
        "TRN2", target_bir_lowering=False, debug=False, enable_asserts=False
    )
    io = {
        "audio": nc.dram_tensor("audio", [T, HF], f32r, kind="ExternalInput").ap(),
        "cif_w": nc.dram_tensor("cif_w", [DCH, 128, HF], f16, kind="ExternalInput").ap(),
        
        "text_w": nc.dram_tensor("text_w", [NOUT // 2 * KCH, 128, 1024], f16, kind="ExternalInput").ap(),
        "text_b": nc.dram_tensor("text_b", [1, OUT], f16, kind="ExternalInput").ap(),
        "ident": nc.dram_tensor("ident", [128, 128], f32, kind="ExternalInput").ap(),
        "ustrict": nc.dram_tensor("ustrict", [12, 12], f32, kind="ExternalInput").ap(),
        "sdiag": nc.dram_tensor("sdiag", [12, 12], f32, kind="ExternalInput").ap(),
        "nm1col": nc.dram_tensor("nm1col", [12, 1], f32, kind="ExternalInput").ap(),
        "nt11": nc.dram_tensor("nt11", [1, 1], f32, kind="ExternalInput").ap(),
        "onesM": nc.dram_tensor("onesM", [1, MP], f16, kind="ExternalInput").ap(),
        "out_h": nc.dram_tensor("out_h", [24, MW, 512], f32, kind="ExternalOutput").ap(),
        "out_pred": nc.dram_tensor("out_pred", [1, 1], f32, kind="ExternalOutput").ap(),
    }
    with tile.TileContext(nc) as tc:
        _emit(nc, tc, tile, mybir, io)
    nc.compile()
    return nc


_NC_CACHE = {}


def make_in_maps(audio_features, num_tokens, cif_w, cif_b, text_w_scaled, text_b):
    ident = np.eye(128, dtype=np.float32)
    ustrict = np.triu(np.ones((12, 12), np.float32), k=1)
    sdiag = np.diag(np.ones(11, np.float32), k=1)

    # prepack cif_w (+bias as last row) into [DCH, 128, HF] fp16 tiles
    cifw_p = np.zeros((DCH, 128, HF), np.float16)
    cw16 = cif_w.astype(np.float16)
    for dc in range(DCH - 1):
        cifw_p[dc] = cw16[128 * dc : 128 * dc + 128]
    cifw_p[DCH - 1, 0:127] = cw16[1152:1279]
    cifw_p[DCH - 1, 127] = cif_b.astype(np.float16)

    # prepack text_w into [NOUT//2 * KCH, 128, 1024] fp16 tiles (linear DMAs)
    tw16 = text_w_scaled.astype(np.float16)
    tw_p = np.zeros((NOUT // 2 * KCH, 128, 1024), np.float16)
    for nb in range(NOUT // 2):
        for k in range(KCH):
            tw_p[nb * KCH + k] = tw16[
                128 * k : 128 * k + 128, 1024 * nb : 1024 * nb + 1024
            ]

    in_maps = []
    for b in range(B):
        nt = np.float32(num_tokens[b])
        in_maps.append(
            {
                "audio": np.ascontiguousarray(audio_features[b]),
                "cif_w": cifw_p,
                "text_w": tw_p,
                "text_b": text_b.astype(np.float16).reshape(1, OUT),
                "ident": ident,
                "ustrict": ustrict,
                "sdiag": sdiag,
                "nm1col": np.full((12, 1), nt - 1, np.float32),
                "nt11": np.full((1, 1), nt, np.float32),
                "onesM": np.ones((1, MP), np.float16),
            }
        )
    return in_maps


def gather_h(tiles):
    """Reassemble the [24, 125, 512] linear output tiles into [M, OUT]."""
    h = np.empty((M, OUT), np.float32)
    for nb in range(4):
        for m in range(MCH):
            for j in range(2):
                n = 2 * nb + j
                h[MW * m : MW * m + MW, 512 * n : 512 * n + 512] = tiles[
                    6 * nb + 2 * m + j
                ]
    return h


def kernel(audio_features, num_tokens, rms_w, cif_w, cif_b, text_w, text_b, max_tokens):
    from concourse.bass_utils import run_bass_kernel_spmd

    audio_features = np.asarray(audio_features, dtype=np.float32)
    num_tokens = np.asarray(num_tokens)
    assert int(max_tokens) == M and audio_features.shape == (B, T, HF)

    # fold rms_w into text_w (pure reassociation of (h/rms*rms_w) @ text_w)
    text_w_scaled = (
        np.asarray(text_w, np.float32) * np.asarray(rms_w, np.float32)[:, None]
    ).astype(np.float32)

    if "nc" not in _NC_CACHE:
        _NC_CACHE["nc"] = build_nc()
    nc = _NC_CACHE["nc"]

    in_maps = make_in_maps(
        audio_features, num_tokens,
        np.asarray(cif_w, np.float32), np.asarray(cif_b, np.float32),
        text_w_scaled, np.asarray(text_b, np.float32),
    )
    res = run_bass_kernel_spmd(nc, in_maps, core_ids=list(range(B)))
    h = np.stack([gather_h(r["out_h"]) for r in res.results], axis=0)
    pred = np.array([r["out_pred"][0, 0] for r in res.results], dtype=np.float32)
    return h, pred


# revision 37
# speedup vs baseline: 1.1096x; 1.0018x over previous
"""Trainium2 Bass kernel for nn_CFormerAdapter (CIF audio adapter).

Sharding: pure data parallelism — one batch element per NeuronCore (B=8).

Per core:
  alphas  = sigmoid(audio[:, -1])                          # [T]
  pred    = sum(alphas)                                    # scalar output
  alphas *= num_tokens / pred
  W       = CIF integrate-and-fire weights                 # [M, T]
  hT      = audio[:, :-1].T @ W.T                          # [D, M]  (TensorE)
  h2T     = cif_w.T @ hT (+bias via ones row)              # [H, M]  (TensorE)
  hnT     = h2T * rsqrt(mean_d(h2T^2) + 1e-6)              # RMSNorm
  out     = hnT.T @ text_w (+bias row) with rms_w folded   # [M, OUT]

The sequential CIF scan is parallelized exactly via a cumulative sum:
with every alpha < 1 (alpha = sigmoid * scale, scale < 0.6 always here),
integrate_t = C_t - floor(C_t) and the fire count is floor(C_t), so the
two nonzero weights per time step (a_int at token j0, remainder at j1) are
reconstructed from C = cumsum(alphas) with a hardware prefix-scan plus
iota-compare masks.  Numerical deviation from the sequential reference is
bounded by the cumsum rounding drift (~1e-4 absolute on weights).

The CIF einsum runs in float32r (~1.6e-4 rel err, full PE rate, even
moving-dim required); the cif/text projections run in float16 (weights are
~0.1-scale, so fp16 costs ~5e-4 rel err vs bf16's 2.3e-3) which halves the
dominant weight DMA.  rms_w is folded into text_w on the host and the 1/rms
scaling is fused into the text-proj PSUM->SBUF copy (valid because 1/rms
varies along the PSUM partition dim).  DMA is spread across both hardware
DGE queues (sync + scalar engines) plus the gpsimd software queue, weights
are host-prepacked into fully linear per-tile DMAs, and dummy bf16 matmuls
keep the PE's HAM activity monitor at full clock while the audio streams in.
Measured: ~187-188us per core on TRN2, absmax rel err ~6.6e-4 vs the fp32
reference.
"""

import numpy as np
from contextlib import ExitStack

B = 8
T = 1500
HF = 1280
D = HF - 1
M = 375
OUT = 4096
NCH = 12           # time chunks of 128 (11*128 + 92)
TLAST = T - 11 * 128
DCH = 10           # d chunks of the 1279 contraction (9*128 + 127)
KCH = 10           # 1280 = 10*128
MP = 376           # on-chip padded token dim (fp32r needs an even moving dim)
MCH = 3            # 375 = 3*125
MW = 125
NOUT = 8           # 4096 = 8*512


def _emit(nc, tc, tile, mybir, io):
    f32 = mybir.dt.float32
    f32r = mybir.dt.float32r
    f16 = mybir.dt.float16
    i32 = mybir.dt.int32
    Alu = mybir.AluOpType
    Act = mybir.ActivationFunctionType

    audio = io["audio"]
    cifw_d = io["cif_w"]
    tw_d = io["text_w"]
    tb_d = io["text_b"]
    ident_d = io["ident"]
    ustrict_d = io["ustrict"]
    nm1_d = io["nm1col"]
    nt_d = io["nt11"]
    out_h = io["out_h"]
    out_pred = io["out_pred"]

    with ExitStack() as ctx:
        pers = ctx.enter_context(tc.tile_pool(name="pers", bufs=1))
        pp = ctx.enter_context(tc.tile_pool(name="pp", bufs=1, space="PSUM"))

        # ---------------- constants ----------------
        ident = pers.tile([128, 128], f32, name="ident")
        nc.gpsimd.dma_start(ident[:], ident_d[:])
        ustrict = pers.tile([12, 12], f32, name="ustrict")
        nc.gpsimd.dma_start(ustrict[:], ustrict_d[:])
        sdiag = pers.tile([12, 12], f32, name="sdiag")
        nc.gpsimd.dma_start(sdiag[:], io["sdiag"][:])
        nm1col = pers.tile([12, 1], f32, name="nm1col")
        nc.gpsimd.dma_start(nm1col[:], nm1_d[:])
        nt11 = pers.tile([1, 1], f32, name="nt11")
        nc.gpsimd.dma_start(nt11[:], nt_d[:])
        ones128 = pers.tile([128, 1], f32, name="ones128")
        nc.vector.memset(ones128[:], 1.0)
        ones128h = pers.tile([128, 1], f16, name="ones128h")
        nc.vector.tensor_copy(ones128h[:], ones128[:])
        onesrow = pers.tile([1, 128], f32, name="onesrow")
        nc.vector.memset(onesrow[:], 1.0)
        onesrowh = pers.tile([1, 128], f16, name="onesrowh")
        nc.vector.tensor_copy(onesrowh[:], onesrow[:])
        iota_i = pers.tile([128, MP], i32, name="iota_i")
        nc.gpsimd.iota(iota_i[:], pattern=[[1, MP]], base=0, channel_multiplier=0)
        iota_f = pers.tile([128, MP], f32, name="iota_f")
        nc.vector.tensor_copy(iota_f[:], iota_i[:])
        # global time index over the [12, 128] scan layout, and the
        # "not the last step" mask used to kill the t=T-1 remainder
        giota_i = pers.tile([12, 128], i32, name="giota_i")
        nc.gpsimd.iota(giota_i[:], pattern=[[1, 128]], base=0, channel_multiplier=128)
        giota_f = pers.tile([12, 128], f32, name="giota_f")
        nc.vector.tensor_copy(giota_f[:], giota_i[:])
        maskv = pers.tile([12, 128], f32, name="maskv")
        nc.vector.tensor_scalar(
            maskv[:], giota_f[:], float(T - 1), None, op0=Alu.not_equal
        )

        twp = ctx.enter_context(tc.tile_pool(name="twp", bufs=16))
        osb = ctx.enter_context(tc.tile_pool(name="osb", bufs=6))
        tball = pers.tile([1, OUT], f16, name="tball")
        nc.gpsimd.dma_start(tball[:], tb_d[:])

        # persistent results of the einsum / cif stages
        outT = [pers.tile([128, MP], f16, name=f"outT{i}") for i in range(DCH)]
        hnT = [pers.tile([128, MP], f16, name=f"hnT{i}") for i in range(KCH)]
        cifw = [pers.tile([128, HF], f16, name=f"cifw{i}") for i in range(DCH)]

        with tc.tile_pool(name="big", bufs=1) as big:
            hsb = [big.tile([128, HF], f32r, name=f"hsb{i}") for i in range(NCH)]
            wt = [big.tile([128, MP], f32r, name=f"wt{i}") for i in range(NCH)]
            for tcn in range(NCH):
                kk = TLAST if tcn == NCH - 1 else 128
                eng = [nc.sync, nc.scalar, nc.gpsimd, nc.sync, nc.scalar,
                       nc.sync, nc.scalar, nc.gpsimd, nc.sync, nc.scalar,
                       nc.sync, nc.scalar][tcn]
                eng.dma_start(
                    hsb[tcn][0:kk, :], audio[128 * tcn : 128 * tcn + kk, :]
                )

            for dc in range(DCH):
                eng = nc.scalar if dc % 3 == 2 else nc.sync
                eng.dma_start(cifw[dc][:], cifw_d[dc])

            # PE warm-up: keep the HAM activity monitor at full clock while
            # the audio DMA streams in, so the real matmuls start warm.
            warm_ps = pp.tile([128, 512], f32, name="warm_ps", tag="tmp2")
            identB = pers.tile([128, 128], mybir.dt.bfloat16, name="identB")
            nc.vector.tensor_copy(identB[:], ident[:])
            junkB = pers.tile([128, 512], mybir.dt.bfloat16, name="junkB")
            nc.vector.memset(junkB[:], 0.5)

            def warm(n):
                for _ in range(n):
                    nc.tensor.matmul(
                        warm_ps[:], identB[:], junkB[:], start=True, stop=True
                    )

            warm(135)

            # ---------------- alphas ----------------
            araw = pers.tile([128, NCH], f32, name="araw")
            nc.vector.memset(araw[:], 0.0)
            for tcn in range(NCH):
                kk = TLAST if tcn == NCH - 1 else 128
                nc.vector.tensor_copy(
                    araw[0:kk, tcn : tcn + 1],
                    hsb[tcn][0:kk, HF - 1 : HF].bitcast(f32),
                )
            sig = pers.tile([128, NCH], f32, name="sig")
            nc.vector.memset(sig[:], 0.0)
            nc.scalar.activation(sig[:, 0 : NCH - 1], araw[:, 0 : NCH - 1], Act.Sigmoid)
            nc.scalar.activation(
                sig[0:TLAST, NCH - 1 : NCH], araw[0:TLAST, NCH - 1 : NCH], Act.Sigmoid
            )

            rsum = pers.tile([128, 1], f32, name="rsum")
            nc.vector.tensor_reduce(
                rsum[:], sig[:], axis=mybir.AxisListType.X, op=Alu.add
            )
            pred_ps = pp.tile([1, 1], f32, name="pred_ps", tag="tmp")
            nc.tensor.matmul(pred_ps[:], ones128[:], rsum[:], start=True, stop=True)
            pred_sb = pers.tile([1, 1], f32, name="pred_sb")
            nc.vector.tensor_copy(pred_sb[:], pred_ps[:])
            nc.gpsimd.dma_start(out_pred[:], pred_sb[:])
            rp = pers.tile([1, 1], f32, name="rp")
            nc.vector.reciprocal(rp[:], pred_sb[:])
            scale11 = pers.tile([1, 1], f32, name="scale11")
            nc.vector.tensor_mul(scale11[:], rp[:], nt11[:])

            sigT_ps = pp.tile([12, 128], f32, name="sigT_ps", tag="tmp")
            nc.tensor.transpose(sigT_ps[:], sig[:], ident[:])
            scol_ps = pp.tile([12, 1], f32, name="scol_ps", tag="tmp2")
            nc.tensor.matmul(
                scol_ps[:], onesrow[0:1, 0:12], scale11[:], start=True, stop=True
            )
            alph = pers.tile([12, 128], f32, name="alph")
            nc.vector.tensor_scalar(alph[:], sigT_ps[:], scol_ps[:], None, op0=Alu.mult)

            # ---------------- cumsum & fire reconstruction ----------------
            cloc = pers.tile([12, 128], f32, name="cloc")
            nc.vector.tensor_tensor_scan(
                cloc[:], alph[:], alph[:], 0.0, op0=Alu.add, op1=Alu.bypass
            )
            offs_ps = pp.tile([12, 1], f32, name="offs_ps", tag="tmp2")
            nc.tensor.matmul(
                offs_ps[:], ustrict[:], cloc[:, 127:128], start=True, stop=True
            )
            cc = pers.tile([12, 128], f32, name="cc")
            nc.vector.tensor_scalar(cc[:], cloc[:], offs_ps[:], None, op0=Alu.add)

            kint = pers.tile([12, 128], i32, name="kint")
            nc.vector.tensor_copy(kint[:], cc[:])
            kf = pers.tile([12, 128], f32, name="kf")
            nc.vector.tensor_copy(kf[:], kint[:])
            kgt = pers.tile([12, 128], f32, name="kgt")
            nc.vector.tensor_tensor(kgt[:], kf[:], cc[:], op=Alu.is_gt)
            kk_t = pers.tile([12, 128], f32, name="kk_t")
            nc.vector.tensor_tensor(kk_t[:], kf[:], kgt[:], op=Alu.subtract)
            frac = pers.tile([12, 128], f32, name="frac")
            nc.vector.tensor_tensor(frac[:], cc[:], kk_t[:], op=Alu.subtract)

            # cross-partition shift of the last column via superdiagonal matmul
            kprev = pers.tile([12, 128], f32, name="kprev")
            nc.vector.tensor_copy(kprev[:, 1:128], kk_t[:, 0:127])
            ksh_ps = pp.tile([12, 1], f32, name="ksh_ps", tag="tmp2")
            nc.tensor.matmul(ksh_ps[:], sdiag[:], kk_t[:, 127:128], start=True, stop=True)
            nc.vector.tensor_copy(kprev[:, 0:1], ksh_ps[:])
            fprev = pers.tile([12, 128], f32, name="fprev")
            nc.vector.tensor_copy(fprev[:, 1:128], frac[:, 0:127])
            fsh_ps = pp.tile([12, 1], f32, name="fsh_ps", tag="tmp2")
            nc.tensor.matmul(fsh_ps[:], sdiag[:], frac[:, 127:128], start=True, stop=True)
            nc.vector.tensor_copy(fprev[:, 0:1], fsh_ps[:])

            fired = pers.tile([12, 128], f32, name="fired")
            nc.vector.tensor_tensor(fired[:], kk_t[:], kprev[:], op=Alu.is_gt)

            # pack rows (32-aligned blocks): 0-11 a_int, 32-43 rem, 64-75 j0, 96-107 j1
            pack = pers.tile([128, 128], f32, name="pack")
            nc.vector.memset(pack[:], 0.0)
            afired = pers.tile([12, 128], f32, name="afired")
            nc.vector.tensor_scalar(
                afired[:], fprev[:], -1.0, 1.0, op0=Alu.mult, op1=Alu.add
            )
            d0 = pers.tile([12, 128], f32, name="d0")
            nc.vector.tensor_tensor(d0[:], afired[:], alph[:], op=Alu.subtract)
            t0 = pers.tile([12, 128], f32, name="t0")
            nc.vector.tensor_tensor(t0[:], d0[:], fired[:], op=Alu.mult)
            nc.vector.tensor_tensor(pack[0:12, :], t0[:], alph[:], op=Alu.add)
            remv = pers.tile([12, 128], f32, name="remv")
            nc.vector.tensor_tensor(remv[:], alph[:], pack[0:12, :], op=Alu.subtract)
            # t = T-1 never writes a remainder -> mask it out
            nc.vector.tensor_tensor(pack[32:44, :], remv[:], maskv[:], op=Alu.mult)
            nc.vector.tensor_scalar(
                pack[64:76, :], kprev[:], nm1col[:], None, op0=Alu.min
            )
            nc.vector.tensor_scalar(
                pack[96:108, :], kk_t[:], nm1col[:], None, op0=Alu.min
            )

            packT_ps = pp.tile([128, 128], f32, name="packT_ps", tag="tmp")
            nc.tensor.transpose(packT_ps[:], pack[:], ident[:])
            pk = pers.tile([128, 128], f32, name="pk")
            nc.vector.tensor_copy(pk[:], packT_ps[:])

            # ---------------- W build (per time chunk) ----------------
            with tc.tile_pool(name="wtmp", bufs=2) as wtmp:
                for tcn in range(NCH):
                    ve = nc.vector
                    w0 = wtmp.tile([128, MP], f32, name="w0", tag="w0")
                    ve.tensor_scalar(
                        w0[:], iota_f[:],
                        pk[:, 64 + tcn : 65 + tcn], pk[:, tcn : tcn + 1],
                        op0=Alu.is_equal, op1=Alu.mult,
                    )
                    w1 = wtmp.tile([128, MP], f32, name="w1", tag="w1")
                    ve.tensor_scalar(
                        w1[:], iota_f[:],
                        pk[:, 96 + tcn : 97 + tcn], pk[:, 32 + tcn : 33 + tcn],
                        op0=Alu.is_equal, op1=Alu.mult,
                    )
                    ve.tensor_tensor(wt[tcn][:], w0[:], w1[:], op=Alu.add)

            warm(12)
            # ---------------- CIF einsum: outT[d, m] = sum_t H[t, d] W[m, t] ----
            with tc.tile_pool(name="mmps", bufs=1, space="PSUM") as mmps:
                for g in range(2):
                    ps_mm = [
                        mmps.tile([128, MP], f32, name=f"mm{i}", tag=f"mm{i}")
                        for i in range(5)
                    ]
                    for tcn in range(NCH):
                        kk = TLAST if tcn == NCH - 1 else 128
                        for i in range(5):
                            dc = 5 * g + i
                            dw = 127 if dc == 9 else 128
                            nc.tensor.matmul(
                                ps_mm[i][0:dw, :],
                                hsb[tcn][0:kk, 128 * dc : 128 * dc + dw],
                                wt[tcn][0:kk, :],
                                start=(tcn == 0),
                                stop=(tcn == NCH - 1),
                            )
                    for i in range(5):
                        dc = 5 * g + i
                        dw = 127 if dc == 9 else 128
                        nc.vector.tensor_copy(outT[dc][0:dw, :], ps_mm[i][0:dw, :])
            # ones row for the cif bias trick (DMA: engines can't start at p=127)
            nc.gpsimd.dma_start(outT[9][127:128, :], io["onesM"][:])

        warm(10)
        # ---------------- cif projection (transposed) + RMS ----------------
        if True:
            with (
                tc.tile_pool(name="cps", bufs=2, space="PSUM") as cps,
                tc.tile_pool(name="cps1", bufs=1, space="PSUM") as cps1,
                tc.tile_pool(name="sqp", bufs=1) as sqp,
            ):
                sqts = []
                ssq_ps = cps1.tile([1, MP], f32, name="ssq_ps", tag="ssq")
                for h2 in range(KCH):
                    ps2 = cps.tile([128, MP], f32, name="ps2", tag="ps2")
                    for dc in range(DCH):
                        nc.tensor.matmul(
                            ps2[:],
                            cifw[dc][:, 128 * h2 : 128 * h2 + 128],
                            outT[dc][:],
                            start=(dc == 0),
                            stop=(dc == DCH - 1),
                        )
                    nc.vector.tensor_copy(hnT[h2][:], ps2[:])
                    sqt = sqp.tile([128, MP], f16, name=f"sqt{h2}", tag=f"sqt{h2}")
                    nc.vector.tensor_tensor(sqt[:], hnT[h2][:], hnT[h2][:], op=Alu.mult)
                    sqts.append(sqt)
                # ssq matmuls emitted after all cif matmuls: the PE stream is
                # not head-of-line blocked waiting on the DVE squares
                for h2 in range(KCH):
                    nc.tensor.matmul(
                        ssq_ps[:], ones128h[:], sqts[h2][:],
                        start=(h2 == 0), stop=(h2 == KCH - 1),
                    )

                eps11 = pers.tile([1, 1], f32, name="eps11")
                nc.vector.memset(eps11[:], 1e-6)
                sqrtv = pers.tile([1, MP], f32, name="sqrtv")
                nc.scalar.activation(
                    sqrtv[:], ssq_ps[:], Act.Sqrt, bias=eps11[:], scale=1.0 / HF
                )
                ones11 = pers.tile([1, 1], f32, name="ones11")
                nc.vector.memset(ones11[:], 1.0)

        # ---------------- text projection ----------------
        tbrep = pers.tile([128, OUT], f16, name="tbrep")
        with tc.tile_pool(name="tps", bufs=1, space="PSUM") as tps:
            for jj in range(NOUT):
                tb_ps = pp.tile([128, 512], f32, name=f"tb_ps{jj}", tag="tmp")
                nc.tensor.matmul(
                    tb_ps[:], onesrowh[:], tball[0:1, 512 * jj : 512 * jj + 512],
                    start=True, stop=True,
                )
                nc.vector.tensor_copy(tbrep[:, 512 * jj : 512 * jj + 512], tb_ps[:])
            # n-blocks of 1024 so text_w DMAs move 4KB contiguous lines
            rinvT = []
            for nb in range(NOUT // 2):
                warm(4)
                ps3 = [
                    [
                        tps.tile([MW, 512], f32, name=f"ps3_{m}_{j}", tag=f"ps3_{m}_{j}")
                        for j in range(2)
                    ]
                    for m in range(MCH)
                ]
                for k in range(KCH):
                    twt = twp.tile([128, 1024], f16, name="twt", tag="twt")
                    eng = nc.sync if k % 2 == 0 else nc.scalar
                    eng.dma_start(twt[:], tw_d[nb * KCH + k])
                    for m in range(MCH):
                        for j in range(2):
                            nc.tensor.matmul(
                                ps3[m][j][:],
                                hnT[k][:, MW * m : MW * m + MW],
                                twt[:, 512 * j : 512 * j + 512],
                                start=(k == 0),
                                stop=(k == KCH - 1),
                            )
                if nb == 0:
                    # transposed 1/rms columns, emitted here so the PE stream
                    # reaches them only after independent text matmuls (the
                    # ACT sqrt they depend on finishes in the meantime)
                    for m in range(MCH):
                        rt_ps = pp.tile([MW, 1], f32, name=f"rt_ps{m}", tag="tmp2")
                        nc.tensor.matmul(
                            rt_ps[:], sqrtv[0:1, MW * m : MW * m + MW], ones11[:],
                            start=True, stop=True,
                        )
                        rt = pers.tile([MW, 1], f32, name=f"rinvT{m}")
                        nc.vector.reciprocal(rt[:], rt_ps[:])
                        rinvT.append(rt)
                for m in range(MCH):
                    for j in range(2):
                        n = 2 * nb + j
                        ot = osb.tile([MW, 512], f32, name="ot", tag="ot")
                        nc.vector.scalar_tensor_tensor(
                            ot[:], ps3[m][j][:], rinvT[m][:],
                            tbrep[0:MW, 512 * n : 512 * n + 512],
                            op0=Alu.mult, op1=Alu.add,
                        )
                        if nb == 3:
                            oeng = nc.sync if (m + j) % 2 == 0 else nc.scalar
                        else:
                            oeng = (nc.gpsimd, nc.gpsimd, nc.sync)[nb]
                        oeng.dma_start(out_h[6 * nb + 2 * m + j], ot[:])


def build_nc():
    import concourse.tile as tile
    from concourse import bacc, mybir

    f32 = mybir.dt.float32
    f32r = mybir.dt.float32r
    f16 = mybir.dt.float16
    nc = bacc.Bacc(
        "TRN2", target_bir_lowering=False, debug=False, enable_asserts=False
    )
    io = {
        "audio": nc.dram_tensor("audio", [T, HF], f32r, kind="ExternalInput").ap(),
        "cif_w": nc.dram_tensor("cif_w", [DCH, 128, HF], f16, kind="ExternalInput").ap(),
        
        "text_w": nc.dram_tensor("text_w", [NOUT // 2 * KCH, 128, 1024], f16, kind="ExternalInput").ap(),
        "text_b": nc.dram_tensor("text_b", [1, OUT], f16, kind="ExternalInput").ap(),
        "ident": nc.dram_tensor("ident", [128, 128], f32, kind="ExternalInput").ap(),
        "ustrict": nc.dram_tensor("ustrict", [12, 12], f32, kind="ExternalInput").ap(),
        "sdiag": nc.dram_tensor("sdiag", [12, 12], f32, kind="ExternalInput").ap(),
        "nm1col": nc.dram_tensor("nm1col", [12, 1], f32, kind="ExternalInput").ap(),
        "nt11": nc.dram_tensor("nt11", [1, 1], f32, kind="ExternalInput").ap(),
        "onesM": nc.dram_tensor("onesM", [1, MP], f16, kind="ExternalInput").ap(),
        "out_h": nc.dram_tensor("out_h", [24, MW, 512], f32, kind="ExternalOutput").ap(),
        "out_pred": nc.dram_tensor("out_pred", [1, 1], f32, kind="ExternalOutput").ap(),
    }
    with tile.TileContext(nc) as tc:
        _emit(nc, tc, tile, mybir, io)
    nc.compile()
    return nc


_NC_CACHE = {}


def make_in_maps(audio_features, num_tokens, cif_w, cif_b, text_w_scaled, text_b):
    ident = np.eye(128, dtype=np.float32)
    ustrict = np.triu(np.ones((12, 12), np.float32), k=1)
    sdiag = np.diag(np.ones(11, np.float32), k=1)

    # prepack cif_w (+bias as last row) into [DCH, 128, HF] fp16 tiles
    cifw_p = np.zeros((DCH, 128, HF), np.float16)
    cw16 = cif_w.astype(np.float16)
    for dc in range(DCH - 1):
        cifw_p[dc] = cw16[128 * dc : 128 * dc + 128]
    cifw_p[DCH - 1, 0:127] = cw16[1152:1279]
    cifw_p[DCH - 1, 127] = cif_b.astype(np.float16)

    # prepack text_w into [NOUT//2 * KCH, 128, 1024] fp16 tiles (linear DMAs)
    tw16 = text_w_scaled.astype(np.float16)
    tw_p = np.zeros((NOUT // 2 * KCH, 128, 1024), np.float16)
    for nb in range(NOUT // 2):
        for k in range(KCH):
            tw_p[nb * KCH + k] = tw16[
                128 * k : 128 * k + 128, 1024 * nb : 1024 * nb + 1024
            ]

    in_maps = []
    for b in range(B):
        nt = np.float32(num_tokens[b])
        in_maps.append(
            {
                "audio": np.ascontiguousarray(audio_features[b]),
                "cif_w": cifw_p,
                "text_w": tw_p,
                "text_b": text_b.astype(np.float16).reshape(1, OUT),
                "ident": ident,
                "ustrict": ustrict,
                "sdiag": sdiag,
                "nm1col": np.full((12, 1), nt - 1, np.float32),
                "nt11": np.full((1, 1), nt, np.float32),
                "onesM": np.ones((1, MP), np.float16),
            }
        )
    return in_maps


def gather_h(tiles):
    """Reassemble the [24, 125, 512] linear output tiles into [M, OUT]."""
    h = np.empty((M, OUT), np.float32)
    for nb in range(4):
        for m in range(MCH):
            for j in range(2):
                n = 2 * nb + j
                h[MW * m : MW * m + MW, 512 * n : 512 * n + 512] = tiles[
                    6 * nb + 2 * m + j
                ]
    return h


def kernel(audio_features, num_tokens, rms_w, cif_w, cif_b, text_w, text_b, max_tokens):
    from concourse.bass_utils import run_bass_kernel_spmd

    audio_features = np.asarray(audio_features, dtype=np.float32)
    num_tokens = np.asarray(num_tokens)
    assert int(max_tokens) == M and audio_features.shape == (B, T, HF)

    # fold rms_w into text_w (pure reassociation of (h/rms*rms_w) @ text_w)
    text_w_scaled = (
        np.asarray(text_w, np.float32) * np.asarray(rms_w, np.float32)[:, None]
    ).astype(np.float32)

    if "nc" not in _NC_CACHE:
        _NC_CACHE["nc"] = build_nc()
    nc = _NC_CACHE["nc"]

    in_maps = make_in_maps(
        audio_features, num_tokens,
        np.asarray(cif_w, np.float32), np.asarray(cif_b, np.float32),
        text_w_scaled, np.asarray(text_b, np.float32),
    )
    res = run_bass_kernel_spmd(nc, in_maps, core_ids=list(range(B)))
    h = np.stack([gather_h(r["out_h"]) for r in res.results], axis=0)
    pred = np.array([r["out_pred"][0, 0] for r in res.results], dtype=np.float32)
    return h, pred


# revision 40
# speedup vs baseline: 1.1259x; 1.0147x over previous
"""Trainium2 Bass kernel for nn_CFormerAdapter (CIF audio adapter).

Sharding: pure data parallelism — one batch element per NeuronCore (B=8).

Per core:
  alphas  = sigmoid(audio[:, -1])                          # [T]
  pred    = sum(alphas)                                    # scalar output
  alphas *= num_tokens / pred
  W       = CIF integrate-and-fire weights                 # [M, T]
  hT      = audio[:, :-1].T @ W.T                          # [D, M]  (TensorE)
  h2T     = cif_w.T @ hT (+bias via ones row)              # [H, M]  (TensorE)
  hnT     = h2T * rsqrt(mean_d(h2T^2) + 1e-6)              # RMSNorm
  out     = hnT.T @ text_w (+bias row) with rms_w folded   # [M, OUT]

The sequential CIF scan is parallelized exactly via a cumulative sum:
with every alpha < 1 (alpha = sigmoid * scale, scale < 0.6 always here),
integrate_t = C_t - floor(C_t) and the fire count is floor(C_t), so the
two nonzero weights per time step (a_int at token j0, remainder at j1) are
reconstructed from C = cumsum(alphas) with a hardware prefix-scan plus
iota-compare masks.  Numerical deviation from the sequential reference is
bounded by the cumsum rounding drift (~1e-4 absolute on weights).

The CIF einsum runs in float32r (~1.6e-4 rel err, full PE rate, even
moving-dim required); the cif/text projections run in float16 (weights are
~0.1-scale, so fp16 costs ~5e-4 rel err vs bf16's 2.3e-3) which halves the
dominant weight DMA.  rms_w is folded into text_w on the host and the 1/rms
scaling is fused into the text-proj PSUM->SBUF copy (valid because 1/rms
varies along the PSUM partition dim).  DMA is spread across both hardware
DGE queues (sync + scalar engines) plus the gpsimd software queue, weights
are host-prepacked into fully linear per-tile DMAs, and dummy bf16 matmuls
keep the PE's HAM activity monitor at full clock while the audio streams in.
Measured: ~187-188us per core on TRN2, absmax rel err ~6.6e-4 vs the fp32
reference.
"""

import numpy as np
from contextlib import ExitStack

B = 8
T = 1500
HF = 1280
D = HF - 1
M = 375
OUT = 4096
NCH = 12           # time chunks of 128 (11*128 + 92)
TLAST = T - 11 * 128
DCH = 10           # d chunks of the 1279 contraction (9*128 + 127)
KCH = 10           # 1280 = 10*128
MP = 376           # on-chip padded token dim (fp32r needs an even moving dim)
MCH = 3            # 375 = 3*125
MW = 125
NOUT = 8           # 4096 = 8*512


def _emit(nc, tc, tile, mybir, io):
    f32 = mybir.dt.float32
    f32r = mybir.dt.float32r
    f16 = mybir.dt.float16
    i32 = mybir.dt.int32
    Alu = mybir.AluOpType
    Act = mybir.ActivationFunctionType

    audio = io["audio"]
    cifw_d = io["cif_w"]
    tw_d = io["text_w"]
    tb_d = io["text_b"]
    ident_d = io["ident"]
    ustrict_d = io["ustrict"]
    nm1_d = io["nm1col"]
    nt_d = io["nt11"]
    out_h = io["out_h"]
    out_pred = io["out_pred"]

    with ExitStack() as ctx:
        pers = ctx.enter_context(tc.tile_pool(name="pers", bufs=1))
        pp = ctx.enter_context(tc.tile_pool(name="pp", bufs=1, space="PSUM"))

        # ---------------- constants ----------------
        ident = pers.tile([128, 128], f32, name="ident")
        nc.gpsimd.dma_start(ident[:], ident_d[:])
        ustrict = pers.tile([12, 12], f32, name="ustrict")
        nc.gpsimd.dma_start(ustrict[:], ustrict_d[:])
        sdiag = pers.tile([12, 12], f32, name="sdiag")
        nc.gpsimd.dma_start(sdiag[:], io["sdiag"][:])
        nm1col = pers.tile([12, 1], f32, name="nm1col")
        nc.gpsimd.dma_start(nm1col[:], nm1_d[:])
        nt11 = pers.tile([1, 1], f32, name="nt11")
        nc.gpsimd.dma_start(nt11[:], nt_d[:])
        ones128 = pers.tile([128, 1], f32, name="ones128")
        nc.vector.memset(ones128[:], 1.0)
        ones128h = pers.tile([128, 1], f16, name="ones128h")
        nc.vector.tensor_copy(ones128h[:], ones128[:])
        onesrow = pers.tile([1, 128], f32, name="onesrow")
        nc.vector.memset(onesrow[:], 1.0)
        onesrowh = pers.tile([1, 128], f16, name="onesrowh")
        nc.vector.tensor_copy(onesrowh[:], onesrow[:])
        iota_i = pers.tile([128, MP], i32, name="iota_i")
        nc.gpsimd.iota(iota_i[:], pattern=[[1, MP]], base=0, channel_multiplier=0)
        iota_f = pers.tile([128, MP], f32, name="iota_f")
        nc.vector.tensor_copy(iota_f[:], iota_i[:])
        # global time index over the [12, 128] scan layout, and the
        # "not the last step" mask used to kill the t=T-1 remainder
        giota_i = pers.tile([12, 128], i32, name="giota_i")
        nc.gpsimd.iota(giota_i[:], pattern=[[1, 128]], base=0, channel_multiplier=128)
        giota_f = pers.tile([12, 128], f32, name="giota_f")
        nc.vector.tensor_copy(giota_f[:], giota_i[:])
        maskv = pers.tile([12, 128], f32, name="maskv")
        nc.vector.tensor_scalar(
            maskv[:], giota_f[:], float(T - 1), None, op0=Alu.not_equal
        )

        twp = ctx.enter_context(tc.tile_pool(name="twp", bufs=16))
        osb = ctx.enter_context(tc.tile_pool(name="osb", bufs=6))
        tball = pers.tile([1, OUT], f16, name="tball")
        nc.gpsimd.dma_start(tball[:], tb_d[:])

        # persistent results of the einsum / cif stages
        outT = [pers.tile([128, MP], f16, name=f"outT{i}") for i in range(DCH)]
        hnT = [pers.tile([128, MP], f16, name=f"hnT{i}") for i in range(KCH)]
        cifw = [pers.tile([128, HF], f16, name=f"cifw{i}") for i in range(DCH)]

        with tc.tile_pool(name="big", bufs=1) as big:
            hsb = [big.tile([128, HF], f32r, name=f"hsb{i}") for i in range(NCH)]
            wt = [big.tile([128, MP], f32r, name=f"wt{i}") for i in range(NCH)]
            for tcn in range(NCH):
                kk = TLAST if tcn == NCH - 1 else 128
                eng = [nc.sync, nc.scalar, nc.gpsimd, nc.sync, nc.scalar,
                       nc.sync, nc.scalar, nc.gpsimd, nc.sync, nc.scalar,
                       nc.sync, nc.scalar][tcn]
                eng.dma_start(
                    hsb[tcn][0:kk, :], audio[128 * tcn : 128 * tcn + kk, :]
                )

            for dc in range(DCH):
                eng = nc.scalar if dc % 3 == 2 else nc.sync
                eng.dma_start(cifw[dc][:], cifw_d[dc])

            # PE warm-up: keep the HAM activity monitor at full clock while
            # the audio DMA streams in, so the real matmuls start warm.
            warm_ps = pp.tile([128, 512], f32, name="warm_ps", tag="tmp2")
            identB = pers.tile([128, 128], mybir.dt.bfloat16, name="identB")
            nc.vector.tensor_copy(identB[:], ident[:])
            junkB = pers.tile([128, 512], mybir.dt.bfloat16, name="junkB")
            nc.vector.memset(junkB[:], 0.5)

            def warm(n):
                for _ in range(n):
                    nc.tensor.matmul(
                        warm_ps[:], identB[:], junkB[:], start=True, stop=True
                    )

            warm(135)

            # ---------------- alphas ----------------
            araw = pers.tile([128, NCH], f32, name="araw")
            nc.vector.memset(araw[:], 0.0)
            for tcn in range(NCH):
                kk = TLAST if tcn == NCH - 1 else 128
                nc.vector.tensor_copy(
                    araw[0:kk, tcn : tcn + 1],
                    hsb[tcn][0:kk, HF - 1 : HF].bitcast(f32),
                )
            sig = pers.tile([128, NCH], f32, name="sig")
            nc.vector.memset(sig[:], 0.0)
            nc.scalar.activation(sig[:, 0 : NCH - 1], araw[:, 0 : NCH - 1], Act.Sigmoid)
            nc.scalar.activation(
                sig[0:TLAST, NCH - 1 : NCH], araw[0:TLAST, NCH - 1 : NCH], Act.Sigmoid
            )

            rsum = pers.tile([128, 1], f32, name="rsum")
            nc.vector.tensor_reduce(
                rsum[:], sig[:], axis=mybir.AxisListType.X, op=Alu.add
            )
            pred_ps = pp.tile([1, 1], f32, name="pred_ps", tag="tmp")
            nc.tensor.matmul(pred_ps[:], ones128[:], rsum[:], start=True, stop=True)
            pred_sb = pers.tile([1, 1], f32, name="pred_sb")
            nc.vector.tensor_copy(pred_sb[:], pred_ps[:])
            nc.gpsimd.dma_start(out_pred[:], pred_sb[:])
            rp = pers.tile([1, 1], f32, name="rp")
            nc.vector.reciprocal(rp[:], pred_sb[:])
            scale11 = pers.tile([1, 1], f32, name="scale11")
            nc.vector.tensor_mul(scale11[:], rp[:], nt11[:])

            sigT_ps = pp.tile([12, 128], f32, name="sigT_ps", tag="tmp")
            nc.tensor.transpose(sigT_ps[:], sig[:], ident[:])
            scol_ps = pp.tile([12, 1], f32, name="scol_ps", tag="tmp2")
            nc.tensor.matmul(
                scol_ps[:], onesrow[0:1, 0:12], scale11[:], start=True, stop=True
            )
            alph = pers.tile([12, 128], f32, name="alph")
            nc.vector.tensor_scalar(alph[:], sigT_ps[:], scol_ps[:], None, op0=Alu.mult)

            # ---------------- cumsum & fire reconstruction ----------------
            cloc = pers.tile([12, 128], f32, name="cloc")
            nc.vector.tensor_tensor_scan(
                cloc[:], alph[:], alph[:], 0.0, op0=Alu.add, op1=Alu.bypass
            )
            offs_ps = pp.tile([12, 1], f32, name="offs_ps", tag="tmp2")
            nc.tensor.matmul(
                offs_ps[:], ustrict[:], cloc[:, 127:128], start=True, stop=True
            )
            cc = pers.tile([12, 128], f32, name="cc")
            nc.vector.tensor_scalar(cc[:], cloc[:], offs_ps[:], None, op0=Alu.add)

            kint = pers.tile([12, 128], i32, name="kint")
            nc.vector.tensor_copy(kint[:], cc[:])
            kf = pers.tile([12, 128], f32, name="kf")
            nc.vector.tensor_copy(kf[:], kint[:])
            kgt = pers.tile([12, 128], f32, name="kgt")
            nc.vector.tensor_tensor(kgt[:], kf[:], cc[:], op=Alu.is_gt)
            kk_t = pers.tile([12, 128], f32, name="kk_t")
            nc.vector.tensor_tensor(kk_t[:], kf[:], kgt[:], op=Alu.subtract)
            frac = pers.tile([12, 128], f32, name="frac")
            nc.vector.tensor_tensor(frac[:], cc[:], kk_t[:], op=Alu.subtract)

            # cross-partition shift of the last column via superdiagonal matmul
            kprev = pers.tile([12, 128], f32, name="kprev")
            nc.vector.tensor_copy(kprev[:, 1:128], kk_t[:, 0:127])
            ksh_ps = pp.tile([12, 1], f32, name="ksh_ps", tag="tmp2")
            nc.tensor.matmul(ksh_ps[:], sdiag[:], kk_t[:, 127:128], start=True, stop=True)
            nc.vector.tensor_copy(kprev[:, 0:1], ksh_ps[:])
            fprev = pers.tile([12, 128], f32, name="fprev")
            nc.vector.tensor_copy(fprev[:, 1:128], frac[:, 0:127])
            fsh_ps = pp.tile([12, 1], f32, name="fsh_ps", tag="tmp2")
            nc.tensor.matmul(fsh_ps[:], sdiag[:], frac[:, 127:128], start=True, stop=True)
            nc.vector.tensor_copy(fprev[:, 0:1], fsh_ps[:])

            fired = pers.tile([12, 128], f32, name="fired")
            nc.vector.tensor_tensor(fired[:], kk_t[:], kprev[:], op=Alu.is_gt)

            # pack rows (32-aligned blocks): 0-11 a_int, 32-43 rem, 64-75 j0, 96-107 j1
            pack = pers.tile([128, 128], f32, name="pack")
            nc.vector.memset(pack[:], 0.0)
            afired = pers.tile([12, 128], f32, name="afired")
            nc.vector.tensor_scalar(
                afired[:], fprev[:], -1.0, 1.0, op0=Alu.mult, op1=Alu.add
            )
            d0 = pers.tile([12, 128], f32, name="d0")
            nc.vector.tensor_tensor(d0[:], afired[:], alph[:], op=Alu.subtract)
            t0 = pers.tile([12, 128], f32, name="t0")
            nc.vector.tensor_tensor(t0[:], d0[:], fired[:], op=Alu.mult)
            nc.vector.tensor_tensor(pack[0:12, :], t0[:], alph[:], op=Alu.add)
            remv = pers.tile([12, 128], f32, name="remv")
            nc.vector.tensor_tensor(remv[:], alph[:], pack[0:12, :], op=Alu.subtract)
            # t = T-1 never writes a remainder -> mask it out
            nc.vector.tensor_tensor(pack[32:44, :], remv[:], maskv[:], op=Alu.mult)
            nc.vector.tensor_scalar(
                pack[64:76, :], kprev[:], nm1col[:], None, op0=Alu.min
            )
            nc.vector.tensor_scalar(
                pack[96:108, :], kk_t[:], nm1col[:], None, op0=Alu.min
            )

            packT_ps = pp.tile([128, 128], f32, name="packT_ps", tag="tmp")
            nc.tensor.transpose(packT_ps[:], pack[:], ident[:])
            pk = pers.tile([128, 128], f32, name="pk")
            nc.vector.tensor_copy(pk[:], packT_ps[:])

            # ---------------- W build (per time chunk) ----------------
            with tc.tile_pool(name="wtmp", bufs=2) as wtmp:
                for tcn in range(NCH):
                    ve = nc.vector
                    w0 = wtmp.tile([128, MP], f32, name="w0", tag="w0")
                    ve.tensor_scalar(
                        w0[:], iota_f[:],
                        pk[:, 64 + tcn : 65 + tcn], pk[:, tcn : tcn + 1],
                        op0=Alu.is_equal, op1=Alu.mult,
                    )
                    w1 = wtmp.tile([128, MP], f32, name="w1", tag="w1")
                    ve.tensor_scalar(
                        w1[:], iota_f[:],
                        pk[:, 96 + tcn : 97 + tcn], pk[:, 32 + tcn : 33 + tcn],
                        op0=Alu.is_equal, op1=Alu.mult,
                    )
                    ve.tensor_tensor(wt[tcn][:], w0[:], w1[:], op=Alu.add)

            warm(12)
            # ---------------- CIF einsum: outT[d, m] = sum_t H[t, d] W[m, t] ----
            with tc.tile_pool(name="mmps", bufs=1, space="PSUM") as mmps:
                for g in range(2):
                    ps_mm = [
                        mmps.tile([128, MP], f32, name=f"mm{i}", tag=f"mm{i}")
                        for i in range(5)
                    ]
                    for tcn in range(NCH):
                        kk = TLAST if tcn == NCH - 1 else 128
                        for i in range(5):
                            dc = 5 * g + i
                            dw = 127 if dc == 9 else 128
                            nc.tensor.matmul(
                                ps_mm[i][0:dw, :],
                                hsb[tcn][0:kk, 128 * dc : 128 * dc + dw],
                                wt[tcn][0:kk, :],
                                start=(tcn == 0),
                                stop=(tcn == NCH - 1),
                            )
                    for i in range(5):
                        dc = 5 * g + i
                        dw = 127 if dc == 9 else 128
                        nc.vector.tensor_copy(outT[dc][0:dw, :], ps_mm[i][0:dw, :])
            # ones row for the cif bias trick (DMA: engines can't start at p=127)
            nc.gpsimd.dma_start(outT[9][127:128, :], io["onesM"][:])

        warm(10)
        # ---------------- cif projection (transposed) + RMS ----------------
        if True:
            with (
                tc.tile_pool(name="cps", bufs=2, space="PSUM") as cps,
                tc.tile_pool(name="cps1", bufs=1, space="PSUM") as cps1,
                tc.tile_pool(name="sqp", bufs=1) as sqp,
            ):
                sqts = []
                ssq_ps = cps1.tile([1, MP], f32, name="ssq_ps", tag="ssq")
                for h2 in range(KCH):
                    ps2 = cps.tile([128, MP], f32, name="ps2", tag="ps2")
                    for dc in range(DCH):
                        nc.tensor.matmul(
                            ps2[:],
                            cifw[dc][:, 128 * h2 : 128 * h2 + 128],
                            outT[dc][:],
                            start=(dc == 0),
                            stop=(dc == DCH - 1),
                        )
                    nc.vector.tensor_copy(hnT[h2][:], ps2[:])
                    sqt = sqp.tile([128, MP], f16, name=f"sqt{h2}", tag=f"sqt{h2}")
                    nc.vector.tensor_tensor(sqt[:], hnT[h2][:], hnT[h2][:], op=Alu.mult)
                    sqts.append(sqt)
                # ssq matmuls emitted after all cif matmuls: the PE stream is
                # not head-of-line blocked waiting on the DVE squares
                for h2 in range(KCH):
                    nc.tensor.matmul(
                        ssq_ps[:], ones128h[:], sqts[h2][:],
                        start=(h2 == 0), stop=(h2 == KCH - 1),
                    )

                eps11 = pers.tile([1, 1], f32, name="eps11")
                nc.vector.memset(eps11[:], 1e-6)
                sqrtv = pers.tile([1, MP], f32, name="sqrtv")
                nc.scalar.activation(
                    sqrtv[:], ssq_ps[:], Act.Sqrt, bias=eps11[:], scale=1.0 / HF
                )
                ones11 = pers.tile([1, 1], f32, name="ones11")
                nc.vector.memset(ones11[:], 1.0)

        # ---------------- text projection ----------------
        tbrep = pers.tile([128, OUT], f16, name="tbrep")
        with tc.tile_pool(name="tps", bufs=1, space="PSUM") as tps:
            for jj in range(NOUT):
                tb_ps = pp.tile([128, 512], f32, name=f"tb_ps{jj}", tag="tmp")
                nc.tensor.matmul(
                    tb_ps[:], onesrowh[:], tball[0:1, 512 * jj : 512 * jj + 512],
                    start=True, stop=True,
                )
                nc.vector.tensor_copy(tbrep[:, 512 * jj : 512 * jj + 512], tb_ps[:])
            # n-blocks of 1024 so text_w DMAs move 4KB contiguous lines
            rinvT = []
            for nb in range(NOUT // 2):
                warm(4)
                ps3 = [
                    [
                        tps.tile([MW, 512], f32, name=f"ps3_{m}_{j}", tag=f"ps3_{m}_{j}")
                        for j in range(2)
                    ]
                    for m in range(MCH)
                ]
                for k in range(KCH):
                    twt = twp.tile([128, 1024], f16, name="twt", tag="twt")
                    eng = nc.sync if k % 2 == 0 else nc.scalar
                    eng.dma_start(twt[:], tw_d[nb * KCH + k])
                    for m in range(MCH):
                        for j in range(2):
                            nc.tensor.matmul(
                                ps3[m][j][:],
                                hnT[k][:, MW * m : MW * m + MW],
                                twt[:, 512 * j : 512 * j + 512],
                                start=(k == 0),
                                stop=(k == KCH - 1),
                            )
                if nb == 0:
                    # transposed 1/rms columns, emitted here so the PE stream
                    # reaches them only after independent text matmuls (the
                    # ACT sqrt they depend on finishes in the meantime)
                    for m in range(MCH):
                        rt_ps = pp.tile([MW, 1], f32, name=f"rt_ps{m}", tag="tmp2")
                        nc.tensor.matmul(
                            rt_ps[:], sqrtv[0:1, MW * m : MW * m + MW], ones11[:],
                            start=True, stop=True,
                        )
                        rt = pers.tile([MW, 1], f32, name=f"rinvT{m}")
                        nc.vector.reciprocal(rt[:], rt_ps[:])
                        rinvT.append(rt)
                for m in range(MCH):
                    for j in range(2):
                        n = 2 * nb + j
                        ot = osb.tile([MW, 512], f32, name="ot", tag="ot")
                        nc.vector.scalar_tensor_tensor(
                            ot[:], ps3[m][j][:], rinvT[m][:],
                            tbrep[0:MW, 512 * n : 512 * n + 512],
                            op0=Alu.mult, op1=Alu.add,
                        )
                        if nb == 3:
                            oeng = nc.sync if (m + j) % 2 == 0 else nc.scalar
                        else:
                            oeng = (nc.gpsimd, nc.gpsimd, nc.sync)[nb]
                        oeng.dma_start(out_h[6 * nb + 2 * m + j], ot[:])


def build_nc():
    import concourse.tile as tile
    from concourse import bacc, mybir

    f32 = mybir.dt.float32
    f32r = mybir.dt.float32r
    f16 = mybir.dt.float16
    nc = bacc.Bacc(
        "TRN2", target_bir_lowering=False, debug=False, enable_asserts=False
    )
    io = {
        "audio": nc.dram_tensor("audio", [T, HF], f32r, kind="ExternalInput").ap(),
        "cif_w": nc.dram_tensor("cif_w", [DCH, 128, HF], f16, kind="ExternalInput").ap(),
        
        "text_w": nc.dram_tensor("text_w", [NOUT // 2 * KCH, 128, 1024], f16, kind="ExternalInput").ap(),
        "text_b": nc.dram_tensor("text_b", [1, OUT], f16, kind="ExternalInput").ap(),
        "ident": nc.dram_tensor("ident", [128, 128], f32, kind="ExternalInput").ap(),
        "ustrict": nc.dram_tensor("ustrict", [12, 12], f32, kind="ExternalInput").ap(),
        "sdiag": nc.dram_tensor("sdiag", [12, 12], f32, kind="ExternalInput").ap(),
        "nm1col": nc.dram_tensor("nm1col", [12, 1], f32, kind="ExternalInput").ap(),
        "nt11": nc.dram_tensor("nt11", [1, 1], f32, kind="ExternalInput").ap(),
        "onesM": nc.dram_tensor("onesM", [1, MP], f16, kind="ExternalInput").ap(),
        "out_h": nc.dram_tensor("out_h", [24, MW, 512], f32, kind="ExternalOutput").ap(),
        "out_pred": nc.dram_tensor("out_pred", [1, 1], f32, kind="ExternalOutput").ap(),
    }
    with tile.TileContext(nc) as tc:
        _emit(nc, tc, tile, mybir, io)
    nc.compile()
    return nc


_NC_CACHE = {}


def make_in_maps(audio_features, num_tokens, cif_w, cif_b, text_w_scaled, text_b):
    ident = np.eye(128, dtype=np.float32)
    ustrict = np.triu(np.ones((12, 12), np.float32), k=1)
    sdiag = np.diag(np.ones(11, np.float32), k=1)

    # prepack cif_w (+bias as last row) into [DCH, 128, HF] fp16 tiles
    cifw_p = np.zeros((DCH, 128, HF), np.float16)
    cw16 = cif_w.astype(np.float16)
    for dc in range(DCH - 1):
        cifw_p[dc] = cw16[128 * dc : 128 * dc + 128]
    cifw_p[DCH - 1, 0:127] = cw16[1152:1279]
    cifw_p[DCH - 1, 127] = cif_b.astype(np.float16)

    # prepack text_w into [NOUT//2 * KCH, 128, 1024] fp16 tiles (linear DMAs)
    tw16 = text_w_scaled.astype(np.float16)
    tw_p = np.zeros((NOUT // 2 * KCH, 128, 1024), np.float16)
    for nb in range(NOUT // 2):
        for k in range(KCH):
            tw_p[nb * KCH + k] = tw16[
                128 * k : 128 * k + 128, 1024 * nb : 1024 * nb + 1024
            ]

    in_maps = []
    for b in range(B):
        nt = np.float32(num_tokens[b])
        in_maps.append(
            {
                "audio": np.ascontiguousarray(audio_features[b]),
                "cif_w": cifw_p,
                "text_w": tw_p,
                "text_b": text_b.astype(np.float16).reshape(1, OUT),
                "ident": ident,
                "ustrict": ustrict,
                "sdiag": sdiag,
                "nm1col": np.full((12, 1), nt - 1, np.float32),
                "nt11": np.full((1, 1), nt, np.float32),
                "onesM": np.ones((1, MP), np.float16),
            }
        )
    return in_maps


def gather_h(tiles):
    """Reassemble the [24, 125, 512] linear output tiles into [M, OUT]."""
    h = np.empty((M, OUT), np.float32)
    for nb in range(4):
        for m in range(MCH):
            for j in range(2):
                n = 2 * nb + j
                h[MW * m : MW * m + MW, 512 * n : 512 * n + 512] = tiles[
                    6 * nb + 2 * m + j
                ]
    return h


def kernel(audio_features, num_tokens, rms_w, cif_w, cif_b, text_w, text_b, max_tokens):
    from concourse.bass_utils import run_bass_kernel_spmd

    audio_features = np.asarray(audio_features, dtype=np.float32)
    num_tokens = np.asarray(num_tokens)
    assert int(max_tokens) == M and audio_features.shape == (B, T, HF)

    # fold rms_w into text_w (pure reassociation of (h/rms*rms_w) @ text_w)
    text_w_scaled = (
        np.asarray(text_w, np.float32) * np.asarray(rms_w, np.float32)[:, None]
    ).astype(np.float32)

    if "nc" not in _NC_CACHE:
        _NC_CACHE["nc"] = build_nc()
    nc = _NC_CACHE["nc"]

    in_maps = make_in_maps(
        audio_features, num_tokens,
        np.asarray(cif_w, np.float32), np.asarray(cif_b, np.float32),
        text_w_scaled, np.asarray(text_b, np.float32),
    )
    res = run_bass_kernel_spmd(nc, in_maps, core_ids=list(range(B)))
    h = np.stack([gather_h(r["out_h"]) for r in res.results], axis=0)
    pred = np.array([r["out_pred"][0, 0] for r in res.results], dtype=np.float32)
    return h, pred
